# revision 1
# baseline (speedup 1.0000x reference)
"""EntityAwareAttention TRN2 Bass kernel — 8-core data parallel.

Problem (per full batch): B=64, L=256, H=1024, P=64, A=512, T=8.
  e1_h/e2_h   = word_hiddens gathered at e1_end/e2_end           [B, H]
  e*_type     = softmax(e_h @ tE.T) @ tE                          [B, H]
  ef          = concat(e1_h, e1_type, e2_h, e2_type)              [B, 4H]
  dense_pos   = concat(wh, pos_e1, pos_e2) @ W_pos                [B, L, A]
  dense_ent   = ef @ W_ent                                        [B, A]
  u           = tanh(dense_pos + repeat-interleave(dense_ent))    [B, L, A]
                (addend for (l, a) is dense_ent[b, 2l + (a>=256)])
  vu          = u @ v ; alpha = softmax(vu, axis=L)               [B, L]
  z           = sum_l alpha[b,l] * wh[b,l,:]                      [B, H]

Sharding: batch across 8 cores (8 batches/core); weights replicated.
The host additionally ships pos_featuresT = concat(wh, p1, p2).T per core
(pure layout prep, no arithmetic) so the contraction dim is already on
partitions for the big matmul.

Per-core structure:
  * tokens t = b*L + l; 16 token-tiles of 128.
  * dense_pos[t_tile] = sum_k whT_k.T @ W_pos_k  (9 fp32r matmuls/tile at
    full PE rate, N=512). fp32r-typed DRAM tensors carry plain fp32 bytes;
    the dtype satisfies the fp32r-producer verifier with no cast ops.
  * dense_ent addend applied as per-partition ACT bias in the tanh
    (halves a<256 / a>=256) from a transposed parity-split dense_ent.
  * vu via DVE mult + reduce; softmax over L through tiny PE transposes
    into an [8, 256] batch-major view.
  * z via block-diagonal alpha: z[8, A-chunk] += alpha_blocked_i.T @ wh_i
    accumulated across all 16 token-tiles in PSUM (zero columns mask
    foreign batches).
"""

import numpy as np
import ml_dtypes

import concourse.bass as bass
import concourse.tile as tile
from concourse import bacc, mybir
from concourse.bass_utils import run_bass_kernel_spmd

F32 = mybir.dt.float32
F32R = mybir.dt.float32r
BF16 = mybir.dt.bfloat16
I32 = mybir.dt.int32
AF = mybir.ActivationFunctionType
ALU = mybir.AluOpType

B, L, H, P2, A, T = 64, 256, 1024, 64, 512, 8
NCORES = 8
BL = B // NCORES            # 8 local batches
TOK = BL * L                # 2048 tokens
NT = TOK // 128             # 16 token tiles
F = H + 2 * P2              # 1152 contraction dim
KF = F // 128               # 9 k-tiles
KE = 4 * H // 128           # 32 W_ent k-tiles
HC = H // 128               # 8 h-chunks


def _build_core(tc):
    nc = tc.nc
    whT_d = nc.dram_tensor("whT", [F, TOK], BF16, kind="ExternalInput").ap()
    wh_d = nc.dram_tensor("word_hiddens", [TOK, H], F32R, kind="ExternalInput").ap()
    whz_d = nc.dram_tensor("wh_bf16", [TOK, H], BF16, kind="ExternalInput").ap()
    e1_d = nc.dram_tensor("e1_end", [BL, 1], I32, kind="ExternalInput").ap()
    e2_d = nc.dram_tensor("e2_end", [BL, 1], I32, kind="ExternalInput").ap()
    te_d = nc.dram_tensor("type_embeddings", [T, H], BF16, kind="ExternalInput").ap()
    wpos_d = nc.dram_tensor("W_pos", [F, A], BF16, kind="ExternalInput").ap()
    went_d = nc.dram_tensor("W_ent", [4 * H, A], BF16, kind="ExternalInput").ap()
    v_d = nc.dram_tensor("v", [1, A], F32, kind="ExternalInput").ap()
    out_d = nc.dram_tensor("out", [BL, H], F32, kind="ExternalOutput").ap()

    const = tc.alloc_tile_pool(name="const", bufs=1)
    whs = tc.alloc_tile_pool(name="whs", bufs=4)
    work = tc.alloc_tile_pool(name="work", bufs=2)
    went_pool = tc.alloc_tile_pool(name="went", bufs=6)
    ps_dp = tc.alloc_tile_pool(name="ps_dp", bufs=3, space="PSUM")
    ps_tr = tc.alloc_tile_pool(name="ps_tr", bufs=2, space="PSUM")
    ps_sm = tc.alloc_tile_pool(name="ps_sm", bufs=3, space="PSUM")

    # ---- gather chain first (Pool FIFO must reach the indirect DMA fast) ----
    ends = const.tile([2 * BL, 1], I32)
    nc.sync.dma_start(ends[0:BL, :], e1_d[:])
    nc.sync.dma_start(ends[BL:2 * BL, :], e2_d[:])
    gidx = const.tile([2 * BL, 1], I32)
    nc.gpsimd.iota(gidx[:], pattern=[[1, 1]], base=0, channel_multiplier=L)
    nc.vector.tensor_scalar(out=gidx[:], in0=gidx[:], scalar1=BL * L - 1,
                            scalar2=None, op0=ALU.bitwise_and)
    nc.vector.tensor_tensor(out=gidx[:], in0=gidx[:], in1=ends[:], op=ALU.add)

    eh = const.tile([2 * BL, H], F32R)
    nc.gpsimd.indirect_dma_start(
        out=eh[:], out_offset=None, in_=wh_d[:],
        in_offset=bass.IndirectOffsetOnAxis(ap=gidx[:, 0:1], axis=0))

    # ---- other constants / small loads ----
    iota_p = const.tile([128, 128], I32)
    iota_f = const.tile([128, 128], I32)
    nc.gpsimd.iota(iota_p[:], pattern=[[0, 128]], base=0, channel_multiplier=1)
    nc.gpsimd.iota(iota_f[:], pattern=[[1, 128]], base=0, channel_multiplier=0)
    ident = const.tile([128, 128], F32R)
    nc.vector.tensor_tensor(out=ident[:], in0=iota_p[:], in1=iota_f[:],
                            op=ALU.is_equal)

    ones = const.tile([128, 1], F32)
    nc.gpsimd.memset(ones[:], 1.0)
    ident_bf = const.tile([128, 128], BF16)
    nc.vector.tensor_copy(out=ident_bf[:], in_=ident[:].bitcast(F32))

    v_sb = const.tile([1, A], F32)
    nc.sync.dma_start(v_sb[:], v_d[:])
    v_bc = const.tile([128, A], F32)
    nc.gpsimd.partition_broadcast(v_bc[:], v_sb[0:1, :])

    te_sb = const.tile([T, H], BF16)
    nc.sync.dma_start(te_sb[:], te_d[:])

    # ---- first dense_pos operands, then W_ent stream ----
    wpos = const.tile([128, KF * A], BF16)
    nc.sync.dma_start(
        wpos.rearrange("p (k a) -> p k a", k=KF)[:, 0:3],
        wpos_d.rearrange("(k p) a -> p k a", p=128)[:, 0:3])
    whT = const.tile([128, NT * F], BF16)
    whT_cols = whT_d.rearrange("(k p) t -> p k t", p=128)
    nc.sync.dma_start(
        whT.rearrange("p (j k c) -> p j k c", j=NT // 2, k=KF)[:, 0],
        whT_cols[:, :, 0:256])

    # ---- W_ent stream (dense_ent gates the tanh bias) ----
    went_tiles = []
    for q in range(KE // 4):
        wt = went_pool.tile([128, 4 * A], BF16, tag="went", name=f"went{q}")
        nc.sync.dma_start(
            wt.rearrange("p (g a) -> p g a", g=4),
            went_d[q * 512:(q + 1) * 512, :].rearrange("(g p) a -> p g a", p=128))
        went_tiles.append(wt)

    # ---- big constant loads (whT block 0 + wpos k0..2 were loaded above) ----
    for g in range(1, 3):
        nc.sync.dma_start(
            wpos.rearrange("p (k a) -> p k a", k=KF)[:, g * 3:(g + 1) * 3],
            wpos_d.rearrange("(k p) a -> p k a", p=128)[:, g * 3:(g + 1) * 3])
    wh_sb = []
    for j in range(NT // 2):
        if j > 0:
            nc.sync.dma_start(
                whT.rearrange("p (j k c) -> p j k c", j=NT // 2, k=KF)[:, j],
                whT_cols[:, :, j * 256:(j + 1) * 256])
        for i in (2 * j, 2 * j + 1):
            wt = whs.tile([128, H], BF16, tag="wh", name=f"wh{i}")
            nc.sync.dma_start(wt[:], whz_d[i * 128:(i + 1) * 128, :])
            wh_sb.append(wt)

    # ---- hoisted dense_pos matmuls for tiles 0-1 (fill the PE head while
    # the gather/entity chain is still in flight) ----
    dp_pre = []
    for i in range(2):
        dp = ps_dp.tile([128, A], F32, tag="dp", name=f"dp_pre{i}")
        for k in range(KF):
            off = (i // 2) * KF * 256 + k * 256 + (i % 2) * 128
            nc.tensor.matmul(
                dp[:], lhsT=whT[:, off: off + 128],
                rhs=wpos[:, k * A:(k + 1) * A],
                start=(k == 0), stop=(k == KF - 1))
        dp_pre.append(dp)

    teT = const.tile([128, HC * T], BF16)
    for hc in range(HC):
        pt = ps_tr.tile([128, 128], F32R, tag="tr")
        ptb = pt.bitcast(BF16)
        nc.tensor.transpose(ptb[:, 0:T], te_sb[:, hc * 128:(hc + 1) * 128],
                            ident_bf[0:T, 0:T])
        nc.vector.tensor_copy(out=teT[:, hc * T:(hc + 1) * T], in_=ptb[:, 0:T])

    # ---- entity features efT[:, kt*8:+8], 32 k-tiles ----
    # regions: 0=e1_h(kt0..7) 1=e2_h(8..15) 2=e1_type(16..23) 3=e2_type(24..31)
    # (W_ent rows are host-permuted to match, so dense_ent can consume the
    # gather-only e_h halves before the softmax chain finishes.)
    efT = const.tile([128, KE * BL], BF16)
    for hc in range(HC):
        pt = ps_tr.tile([128, 128], F32R, tag="tr")
        nc.tensor.transpose(pt[:, 0:2 * BL], eh[:, hc * 128:(hc + 1) * 128],
                            ident[0:2 * BL, 0:2 * BL])
        nc.vector.tensor_copy(out=efT[:, hc * BL:(hc + 1) * BL],
                              in_=pt[:, 0:BL].bitcast(F32))
        nc.vector.tensor_copy(
            out=efT[:, (HC + hc) * BL:(HC + hc + 1) * BL],
            in_=pt[:, BL:2 * BL].bitcast(F32))

    for ent in range(2):
        sc = ps_sm.tile([BL, T], F32, tag="sm")
        for hc in range(HC):
            col = (0 if ent == 0 else HC) + hc
            nc.tensor.matmul(sc[:], lhsT=efT[:, col * BL:(col + 1) * BL],
                             rhs=teT[:, hc * T:(hc + 1) * T],
                             start=(hc == 0), stop=(hc == HC - 1))
        asm = const.tile([BL, T], F32, tag=f"asm{ent}")
        ssum = const.tile([BL, 1], F32, tag=f"ssum{ent}")
        nc.scalar.activation(asm[:], sc[:], AF.Exp, accum_out=ssum[:])
        rs = const.tile([BL, 1], F32, tag=f"rs{ent}")
        nc.vector.reciprocal(rs[:], ssum[:])
        al = const.tile([BL, T], F32R, tag=f"al{ent}")
        nc.vector.tensor_scalar(out=al[:], in0=asm[:], scalar1=rs[:, 0:1],
                                scalar2=None, op0=ALU.mult)
        pt = ps_tr.tile([128, 128], F32R, tag="tr")
        nc.tensor.transpose(pt[0:T, 0:BL], al[:],
                            ident[0:BL, 0:BL])
        alTe = const.tile([T, BL], BF16, tag=f"alTe{ent}")
        nc.vector.tensor_copy(out=alTe[:], in_=pt[0:T, 0:BL].bitcast(F32))
        for hc in range(HC):
            pe = ps_sm.tile([128, BL], F32, tag="sm")
            nc.tensor.matmul(pe[:], lhsT=te_sb[:, hc * 128:(hc + 1) * 128],
                             rhs=alTe[:], start=True, stop=True)
            col = (2 * HC if ent == 0 else 3 * HC) + hc
            nc.vector.tensor_copy(out=efT[:, col * BL:(col + 1) * BL],
                                  in_=pe[:])

    # ---- dense_ent matmuls (W_ent tiles DMA'd up front) ----
    de = ps_sm.tile([BL, A], F32, tag="sm")
    for k in range(KE):
        nc.tensor.matmul(de[:], lhsT=efT[:, k * BL:(k + 1) * BL],
                         rhs=went_tiles[k // 4][:, (k % 4) * A:(k % 4 + 1) * A],
                         start=(k == 0), stop=(k == KE - 1))

    # parity split (even a's then odd a's) + transpose to [l(p), b] bias cols
    de_eo = const.tile([BL, A], F32R)
    nc.vector.tensor_copy(
        out=de_eo.rearrange("b (two l) -> b two l", two=2),
        in_=de.rearrange("b (l two) -> b two l", two=2))
    # bias_sb cols: parity*16 + half*8 + b
    bias_sb = const.tile([128, 32], F32)
    for par in range(2):
        for half in range(2):
            pt = ps_tr.tile([128, 128], F32R, tag="tr")
            src = de_eo[:, par * 256 + half * 128: par * 256 + (half + 1) * 128]
            nc.tensor.transpose(pt[:, 0:BL], src, ident[0:BL, 0:BL])
            nc.vector.tensor_copy(
                out=bias_sb[:, par * 16 + half * 8: par * 16 + half * 8 + BL],
                in_=pt[:, 0:BL].bitcast(F32))

    # ---- main loop over token tiles ----
    # Unnormalized attention: w = exp(vu) accumulates into z immediately
    # (block-diagonal matmul); normalization by 1/sum(exp) happens once at
    # the end. exp args are bounded (|vu| <= sum|v| ~ 25) so no max-shift.
    vu0 = const.tile([128, BL], F32)     # vu for l in [0,128), col = b
    vu1 = const.tile([128, BL], F32)     # vu for l in [128,256), col = b
    expc0 = const.tile([128, BL], F32)   # exp(vu) same layout
    expc1 = const.tile([128, BL], F32)
    alblk = const.tile([128, NT * BL], BF16)
    nc.gpsimd.memset(alblk[:], 0.0)
    zp0 = ps_sm.tile([BL, A], F32, tag="sm")
    zp1 = ps_sm.tile([BL, A], F32, tag="sm")
    for i in range(NT):
        b, half = i // 2, i % 2
        if i < 2:
            dp = dp_pre[i]
        else:
            dp = ps_dp.tile([128, A], F32, tag="dp")
            for k in range(KF):
                off = (i // 2) * KF * 256 + k * 256 + (i % 2) * 128
                nc.tensor.matmul(
                    dp[:],
                    lhsT=whT[:, off: off + 128],
                    rhs=wpos[:, k * A:(k + 1) * A],
                    start=(k == 0), stop=(k == KF - 1))
        u = work.tile([128, A], F32, tag="u")
        nc.scalar.activation(u[:, 0:256], dp[:, 0:256], AF.Tanh,
                             bias=bias_sb[:, half * 8 + b: half * 8 + b + 1])
        nc.scalar.activation(u[:, 256:512], dp[:, 256:512], AF.Tanh,
                             bias=bias_sb[:, 16 + half * 8 + b: 16 + half * 8 + b + 1])
        scr = work.tile([128, A], F32, tag="scr")
        vu_dst = (vu0 if half == 0 else vu1)
        nc.vector.tensor_tensor(out=scr[:], in0=u[:], in1=v_bc[:], op=ALU.mult)
        nc.vector.tensor_reduce(out=vu_dst[:, b:b + 1], in_=scr[:],
                                axis=mybir.AxisListType.X, op=ALU.add)
        exp_dst = (expc0 if half == 0 else expc1)
        nc.scalar.activation(exp_dst[:, b:b + 1], vu_dst[:, b:b + 1], AF.Exp)
        nc.vector.tensor_copy(out=alblk[:, i * BL + b: i * BL + b + 1],
                              in_=exp_dst[:, b:b + 1])
        # z matmuls are deferred by one iteration: tile i-1's z runs after
        # tile i's dense_pos matmuls so the PE never stalls on the
        # tanh->vu->exp chain of the tile it just produced.
        if i > 0:
            for chunk, zp in ((0, zp0), (1, zp1)):
                nc.tensor.matmul(zp[:],
                                 lhsT=alblk[:, (i - 1) * BL:i * BL],
                                 rhs=wh_sb[i - 1][:, chunk * A:(chunk + 1) * A],
                                 start=(i == 1), stop=False)

    for chunk, zp in ((0, zp0), (1, zp1)):
        nc.tensor.matmul(zp[:],
                         lhsT=alblk[:, (NT - 1) * BL:NT * BL],
                         rhs=wh_sb[NT - 1][:, chunk * A:(chunk + 1) * A],
                         start=False, stop=True)

    # ---- normalization epilogue: esum via ones-matmul ----
    ecs = const.tile([128, BL], F32)
    nc.vector.tensor_tensor(out=ecs[:], in0=expc0[:], in1=expc1[:], op=ALU.add)
    esp = ps_tr.tile([128, 128], F32, tag="tr")
    nc.tensor.matmul(esp[0:BL, 0:1], lhsT=ecs[:], rhs=ones[:],
                     start=True, stop=True)
    ers = const.tile([BL, 1], F32)
    nc.vector.reciprocal(ers[:], esp[0:BL, 0:1])
    # normalize the two chunks on different engines so they run in parallel
    z_sb = const.tile([BL, H], F32)
    nc.scalar.activation(z_sb[:, 0:A], zp0[:], AF.Copy, scale=ers[:, 0:1])
    nc.vector.tensor_scalar(out=z_sb[:, A:H], in0=zp1[:],
                            scalar1=ers[:, 0:1], scalar2=None, op0=ALU.mult)

    nc.sync.dma_start(out_d[:], z_sb[:])

    for p in (ps_sm, ps_tr, ps_dp, went_pool, work, whs, const):
        p.release()


def build():
    nc = bacc.Bacc("TRN2", target_bir_lowering=False, debug=False,
                   num_devices=NCORES)
    with tile.TileContext(nc) as tc:
        _build_core(tc)
    nc.compile()
    return nc


_NC = None


def kernel(word_hiddens, pos_e1_embeddings, pos_e2_embeddings, e1_end, e2_end,
           type_embeddings, W_pos, W_ent, v):
    global _NC
    if _NC is None:
        _NC = build()
    wh = np.ascontiguousarray(word_hiddens, dtype=np.float32).reshape(B, L, H)
    p1 = np.ascontiguousarray(pos_e1_embeddings, dtype=np.float32).reshape(B, L, P2)
    p2 = np.ascontiguousarray(pos_e2_embeddings, dtype=np.float32).reshape(B, L, P2)
    e1 = np.asarray(e1_end, dtype=np.int32).reshape(B)
    e2 = np.asarray(e2_end, dtype=np.int32).reshape(B)
    te = np.ascontiguousarray(type_embeddings, dtype=np.float32).astype(ml_dtypes.bfloat16)
    wp = np.ascontiguousarray(W_pos, dtype=np.float32).astype(ml_dtypes.bfloat16)
    we0 = np.asarray(W_ent, dtype=np.float32).reshape(4, H, A)
    we = np.ascontiguousarray(
        np.concatenate([we0[0], we0[2], we0[1], we0[3]],
                       axis=0)).astype(ml_dtypes.bfloat16)
    vv = np.ascontiguousarray(v, dtype=np.float32).reshape(1, A)

    in_maps = []
    for c in range(NCORES):
        s = slice(c * BL, (c + 1) * BL)
        whc = np.ascontiguousarray(wh[s].reshape(TOK, H))
        pf = np.empty((TOK, F), dtype=np.float32)
        pf[:, :H] = whc
        pf[:, H:H + P2] = p1[s].reshape(TOK, P2)
        pf[:, H + P2:] = p2[s].reshape(TOK, P2)
        in_maps.append({
            "whT": np.ascontiguousarray(pf.T).astype(ml_dtypes.bfloat16),
            "word_hiddens": whc,
            "wh_bf16": whc.astype(ml_dtypes.bfloat16),
            "e1_end": e1[s].reshape(BL, 1),
            "e2_end": e2[s].reshape(BL, 1),
            "type_embeddings": te,
            "W_pos": wp,
            "W_ent": we,
            "v": vv,
        })
    res = run_bass_kernel_spmd(_NC, in_maps, core_ids=list(range(NCORES)))
    return np.concatenate([res.results[c]["out"] for c in range(NCORES)], axis=0)



# revision 14
# speedup vs baseline: 1.4689x; 1.4689x over previous
"""EntityAwareAttention TRN2 Bass kernel — 8-core data parallel, v2.

Cost-model-driven redesign vs baseline:
  * dense_ent computed TRANSPOSED (deT[a_perm, b]): lhsT = W_ent' k-tile
    (stationary, free), rhs = efT [128, 8] -> out [128, 8] = 8 cycles/instr
    instead of out [8, 512] = 512 cycles. W_ent columns are host-permuted so
    deT slices ARE the per-partition tanh-bias columns directly (one copy).
  * z computed TRANSPOSED (zT[h, b]): lhsT = wh tile h-slice (stationary),
    rhs = exp column [128, 1] -> out [128, 1] = 1 cycle/instr, accumulated
    per batch across its two token tiles. Replaces 16.4K PE cycles.
  * vu via ONE fused DVE tensor_tensor_reduce per tile (bf16 operands).
  * exp kept unnormalized; esum via two strided-column ones-matmuls.
  * PE "heater": dummy matmuls at the head keep the PE p-state ramp warm
    while the first DMAs land (cold PE runs at 1.2GHz for 3us).
  * W2/W4 (the e*_type halves of W_ent) shipped fp8-e4m3 (error-neutral:
    type features are ~3% of dense_ent magnitude).
  * All weights resident in SBUF (no buffer recycling); DMAs are few, large,
    partition-major-contiguous, and ordered so the PE pipeline never waits.

Numerics: rel-err budget is 2e-2; this design measures ~2.6e-3 in numpy.
"""

import numpy as np
import ml_dtypes

import concourse.bass as bass
import concourse.tile as tile
from concourse import bacc, mybir
from concourse.bass_utils import run_bass_kernel_spmd

F32 = mybir.dt.float32
F32R = mybir.dt.float32r
BF16 = mybir.dt.bfloat16
FP8 = mybir.dt.float8e4
I32 = mybir.dt.int32
AF = mybir.ActivationFunctionType
ALU = mybir.AluOpType

B, L, H, P2, A, T = 64, 256, 1024, 64, 512, 8
NCORES = 8
BL = B // NCORES            # 8 local batches
TOK = BL * L                # 2048 tokens
NT = TOK // 128             # 16 token tiles
F = H + 2 * P2              # 1152 contraction dim
KF = F // 128               # 9 k-tiles for dense_pos
KB = 7                      # bf16 k-tiles; k7+k8 go fp8 DoubleRow
KE1 = 16                    # W1;W3 k-tiles (e_h halves)
KE2 = 16                    # W2;W4 k-tiles (type halves)
HC = H // 128               # 8 h-chunks

# ---- tuning knobs (sim-derived) ----
N_HEAT = 70                 # heater matmuls at the head
MID_HEAT = 75               # heater matmuls between dp0 k2 and k3 (wpos wait)
HEAT_COLS = 64
LAG = 8                     # tanh for tile j gated at dp of tile j+LAG
NCOPY = 8                   # dp tiles 0..NCOPY-1 copied PSUM->SBUF (bias wait)
import os as _os
E_GATHER = int(_os.environ.get("E_GATHER", 0))
E_TYPE = int(_os.environ.get("E_TYPE", 1))
E_DE = int(_os.environ.get("E_DE", 5))


def _build_core(tc):
    nc = tc.nc
    whT_d = nc.dram_tensor("whT", [128, NT * KB * 128], BF16, kind="ExternalInput").ap()
    whT8_d = nc.dram_tensor("whT8", [128, NT * 2 * 128], FP8, kind="ExternalInput").ap()
    whz_d = nc.dram_tensor("whz", [128, NT * H], BF16, kind="ExternalInput").ap()
    wpos_d = nc.dram_tensor("W_pos", [128, KB * A], BF16, kind="ExternalInput").ap()
    wpos8_d = nc.dram_tensor("W_pos8", [128, 2 * A], FP8, kind="ExternalInput").ap()
    went1_d = nc.dram_tensor("went1", [128, KE1 * A], BF16, kind="ExternalInput").ap()
    went2_d = nc.dram_tensor("went2", [128, KE2 * A], FP8, kind="ExternalInput").ap()
    wh_d = nc.dram_tensor("word_hiddens", [TOK, H], F32R, kind="ExternalInput").ap()
    ends_d = nc.dram_tensor("ends", [2 * BL, 1], I32, kind="ExternalInput").ap()
    te_d = nc.dram_tensor("type_embeddings", [T, H], BF16, kind="ExternalInput").ap()
    v_d = nc.dram_tensor("v", [1, A], BF16, kind="ExternalInput").ap()
    out_d = nc.dram_tensor("out", [HC * 128, BL], F32, kind="ExternalOutput").ap()

    const = tc.alloc_tile_pool(name="const", bufs=1)
    work = tc.alloc_tile_pool(name="work", bufs=3)
    ps_dp = tc.alloc_tile_pool(name="ps_dp", bufs=5, space="PSUM")
    ps_sm = tc.alloc_tile_pool(name="ps_sm", bufs=2, space="PSUM")
    ps_acc = tc.alloc_tile_pool(name="ps_acc", bufs=1, space="PSUM")

    # ---------- heater constants (Pool engine, ready ~0.5us) ----------
    hl = const.tile([128, HEAT_COLS], BF16, name="hl")
    nc.gpsimd.memset(hl[:], 0.0)

    # ---------- DMA schedule ----------
    # SP queue: the big weight stream, most-urgent first.
    whT = const.tile([128, NT * KB * 128], BF16, name="whT")
    whT8 = const.tile([128, NT * 2 * 128], FP8, name="whT8")
    wpos = const.tile([128, KB * A], BF16, name="wpos")
    wpos8 = const.tile([128, 2 * A], FP8, name="wpos8")
    whz = const.tile([128, NT * H], BF16, name="whz")
    went1 = const.tile([128, KE1 * A], BF16, name="went1")
    went2 = const.tile([128, KE2 * A], FP8, name="went2")

    CW = KB * 128  # whT bf16 columns per token tile

    def whT_dma(lo, hi):
        nc.sync.dma_start(whT[:, lo * CW:hi * CW], whT_d[:, lo * CW:hi * CW])

    # small loads first: tiny transfers, but the gather chain hangs off ends
    # (ACT-issued DMA faults the device, so everything goes via SP)
    ends = const.tile([2 * BL, 1], I32, name="ends")
    nc.sync.dma_start(ends[:], ends_d[:])
    te_sb = const.tile([T, H], BF16, name="te_sb")
    nc.sync.dma_start(te_sb[:], te_d[:])
    v_sb = const.tile([1, A], BF16, name="v_sb")
    nc.sync.dma_start(v_sb[:], v_d[:])
    nc.sync.dma_start(wpos[:, 0:3 * A], wpos_d[:, 0:3 * A])              # k0-2
    whT_dma(0, 1)
    nc.sync.dma_start(wpos[:, 3 * A:], wpos_d[:, 3 * A:])                # k3-6
    nc.sync.dma_start(wpos8[:], wpos8_d[:])
    nc.sync.dma_start(whT8[:], whT8_d[:])
    whT_dma(1, 2)
    whT_dma(2, 3)
    whT_dma(3, 4)
    import os
    _ORD = os.environ.get("DMA_ORD", "F")
    went_dmas = [
        lambda: nc.sync.dma_start(went1[:, 0:8 * A], went1_d[:, 0:8 * A]),
        lambda: nc.sync.dma_start(went1[:, 8 * A:], went1_d[:, 8 * A:]),
        lambda: nc.sync.dma_start(went2[:], went2_d[:]),
    ]
    whz_dmas = [
        lambda: nc.sync.dma_start(whz[:, 0:4 * H], whz_d[:, 0:4 * H]),
        lambda: nc.sync.dma_start(whz[:, 4 * H:8 * H], whz_d[:, 4 * H:8 * H]),
        lambda: nc.sync.dma_start(whz[:, 8 * H:12 * H], whz_d[:, 8 * H:12 * H]),
        lambda: nc.sync.dma_start(whz[:, 12 * H:], whz_d[:, 12 * H:]),
    ]
    whT_chunks = [lambda lo=lo: whT_dma(lo, lo + 2) for lo in range(4, 16, 2)]
    # each config: list of ('T', i) / ('E', i) / ('Z', i)
    ORDS = {
        # went all first (prev)
        "A": ["E0", "E1", "E2", "T0", "T1", "T2", "T3", "T4", "T5",
               "Z0", "Z1", "Z2", "Z3"],
        # went interleaved every other whT chunk
        "B": ["E0", "T0", "E1", "T1", "E2", "T2", "T3", "T4", "T5",
               "Z0", "Z1", "Z2", "Z3"],
        # went slightly later
        "C": ["T0", "E0", "T1", "E1", "T2", "E2", "T3", "T4", "T5",
               "Z0", "Z1", "Z2", "Z3"],
        # went2 first (group flip not needed: de_ty start flag handles)
        "D": ["E2", "T0", "E0", "T1", "E1", "T2", "T3", "T4", "T5",
               "Z0", "Z1", "Z2", "Z3"],
        # whz earlier, went mid
        "E": ["E0", "T0", "E1", "T1", "E2", "T2", "T3", "Z0", "T4",
               "Z1", "T5", "Z2", "Z3"],
        # big whT head: tiles 4-7 before went; rest JIT after
        "F": ["T0", "T1", "E0", "E1", "E2", "T2", "T3", "T4", "T5",
               "Z0", "Z1", "Z2", "Z3"],
        # same + whz interleaved among tail whT
        "G": ["T0", "T1", "E0", "E1", "E2", "T2", "Z0", "T3", "Z1",
               "T4", "Z2", "T5", "Z3"],
        # even bigger head
        "H": ["T0", "T1", "T2", "E0", "E1", "E2", "T3", "T4", "T5",
               "Z0", "Z1", "Z2", "Z3"],
    }
    for tok in ORDS[_ORD]:
        kind, idx = tok[0], int(tok[1:])
        if kind == "T":
            whT_chunks[idx]()
        elif kind == "E":
            went_dmas[idx]()
        else:
            whz_dmas[idx]()



    # ---------- gather chain (gpsimd/DVE; latency-critical) ----------
    gidx = const.tile([2 * BL, 1], I32, name="gidx")
    nc.gpsimd.iota(gidx[:], pattern=[[1, 1]], base=0, channel_multiplier=L)
    nc.vector.tensor_scalar(out=gidx[:], in0=gidx[:], scalar1=BL * L - 1,
                            scalar2=None, op0=ALU.bitwise_and)
    nc.vector.tensor_tensor(out=gidx[:], in0=gidx[:], in1=ends[:], op=ALU.add)
    eh = const.tile([2 * BL, H], F32R, name="eh")
    nc.gpsimd.indirect_dma_start(
        out=eh[:], out_offset=None, in_=wh_d[:],
        in_offset=bass.IndirectOffsetOnAxis(ap=gidx[:, 0:1], axis=0))

    # ---------- other small device constants ----------
    iota_p = const.tile([128, 128], I32, name="iota_p")
    iota_f = const.tile([128, 128], I32, name="iota_f")
    nc.gpsimd.iota(iota_p[:], pattern=[[0, 128]], base=0, channel_multiplier=1)
    nc.gpsimd.iota(iota_f[:], pattern=[[1, 128]], base=0, channel_multiplier=0)
    ident = const.tile([128, 128], F32R, name="ident")
    nc.vector.tensor_tensor(out=ident[:], in0=iota_p[:], in1=iota_f[:],
                            op=ALU.is_equal)
    ident_bf = const.tile([128, 128], BF16, name="ident_bf")
    nc.vector.tensor_copy(out=ident_bf[:], in_=ident[:].bitcast(F32))
    ones_bf = const.tile([128, 1], BF16, name="ones_bf")
    nc.gpsimd.memset(ones_bf[:], 1.0)
    v_bc = const.tile([128, A], BF16, name="v_bc")
    nc.gpsimd.partition_broadcast(v_bc[:], v_sb[0:1, :])

    # ---------- PE program ----------
    # 0) heater: keep PE busy (and ramping) until real operands land.
    heat_ps = ps_sm.tile([128, HEAT_COLS], F32, name="heat_ps", tag="sm")
    for _ in range(N_HEAT):
        nc.tensor.matmul(heat_ps[0:HEAT_COLS, :], lhsT=hl[:], rhs=hl[:],
                         start=True, stop=True, skip_group_check=True)

    whT_v = whT.rearrange("p (i k c) -> p i k c", i=NT, k=KB)
    whT8_v = whT8.rearrange("p (i two c) -> p i two c", i=NT, two=2)
    wpos8_v = wpos8.rearrange("p (two a) -> p two a", two=2)
    wpos_v = wpos.rearrange("p (k a) -> p k a", k=KB)
    whz_v = whz.rearrange("p (i h) -> p i h", i=NT)
    went1_v = went1.rearrange("p (k a) -> p k a", k=KE1)
    went2_v = went2.rearrange("p (k a) -> p k a", k=KE2)

    teT = const.tile([128, HC * T], BF16, name="teT")
    efT = const.tile([128, 32 * BL], BF16, name="efT")
    acc = ps_acc.tile([128, 128], F32, name="acc", tag="acc")
    deT = acc[:, 0:4 * BL]
    zt = acc[:, 32:32 + HC * BL]
    esum_row = acc[0:1, 96:96 + BL]
    bias_sb = const.tile([128, 4 * BL], F32, name="bias_sb")
    vu = const.tile([128, NT], F32, name="vu")
    expb = const.tile([128, NT], BF16, name="expb")
    u_t = [None] * NT
    dp_t = [None] * NT

    steps = []          # (min_tile, thunk) — drained between dp k-matmuls

    def emit_dp(i, drain):
        dp = ps_dp.tile([128, A], F32, tag="dp", name=f"dp{i}")
        for k in range(KB):
            nc.tensor.matmul(dp[:], lhsT=whT_v[:, i, k, :], rhs=wpos_v[:, k, :],
                             start=(k == 0), stop=False)
            if i == 0 and k == 2:
                for _ in range(MID_HEAT):
                    nc.tensor.matmul(heat_ps[0:HEAT_COLS, :], lhsT=hl[:],
                                     rhs=hl[:], start=True, stop=True,
                                     skip_group_check=True)
            drain(i)
        if _os.environ.get("NODR"):
            nc.tensor.matmul(dp[:], lhsT=whT8_v[:, i, 0, :], rhs=wpos8_v[:, 0, :],
                             start=False, stop=False)
            nc.tensor.matmul(dp[:], lhsT=whT8_v[:, i, 1, :], rhs=wpos8_v[:, 1, :],
                             start=False, stop=True)
        else:
            nc.tensor.matmul(dp[:], lhsT=whT8_v[:, i, :, :], rhs=wpos8_v[:],
                             start=False, stop=True,
                             perf_mode=mybir.MatmulPerfMode.DoubleRow)
        drain(i)
        if i < NCOPY:
            dpc = const.tile([128, A], F32, name=f"dpc{i}")
            if i % 2 == 0:
                nc.vector.tensor_copy(out=dpc[:], in_=dp[:])
            else:
                nc.scalar.copy(dpc[:], dp[:])
            dp_t[i] = dpc
        else:
            dp_t[i] = dp

    def queue_entity_gather():
        def tr(hc):
            def f():
                pt = ps_sm.tile([128, 128], F32R, tag="sm", name=f"ehT{hc}")
                nc.tensor.transpose(pt[:, 0:2 * BL], eh[:, hc * 128:(hc + 1) * 128],
                                    ident[0:2 * BL, 0:2 * BL])
                nc.vector.tensor_copy(out=efT[:, hc * BL:(hc + 1) * BL],
                                      in_=pt[:, 0:BL].bitcast(F32))
                nc.vector.tensor_copy(out=efT[:, (HC + hc) * BL:(HC + hc + 1) * BL],
                                      in_=pt[:, BL:2 * BL].bitcast(F32))
            return f
        def trte(hc):
            def f():
                pt = ps_sm.tile([128, 128], F32R, tag="sm", name=f"teT{hc}")
                ptb = pt.bitcast(BF16)
                nc.tensor.transpose(ptb[:, 0:T], te_sb[:, hc * 128:(hc + 1) * 128],
                                    ident_bf[0:T, 0:T])
                nc.vector.tensor_copy(out=teT[:, hc * T:(hc + 1) * T],
                                      in_=ptb[:, 0:T])
            return f
        for hc in range(HC):
            steps.append((E_GATHER, tr(hc)))
        for hc in range(HC):
            steps.append((E_GATHER, trte(hc)))

    def queue_de(kts, base, w_v, min_tile):
        # dense_ent k-tiles: 2 matmuls per step; every column region resets
        # on the overall first k-tile and closes on the overall last one.
        def mk(kt, s0):
            def f():
                for s in (s0, s0 + 1):
                    nc.tensor.matmul(deT[:, s * BL:(s + 1) * BL],
                                     lhsT=w_v[:, kt, s * 128:(s + 1) * 128],
                                     rhs=efT[:, (base + kt) * BL:(base + kt + 1) * BL],
                                     start=(base + kt == 0 and s == 0),
                                     stop=(base + kt == KE1 + KE2 - 1 and s == 3),
                                     skip_group_check=True)
            return f
        for kt in range(kts):
            steps.append((min_tile, mk(kt, 0)))
            steps.append((min_tile, mk(kt, 2)))

    def queue_type_chain():
        def sc_mk(ent, hc0):
            def f():
                sc = sc_t[ent]
                for hc in (hc0, hc0 + 1):
                    col = (0 if ent == 0 else HC) + hc
                    nc.tensor.matmul(sc[:], lhsT=efT[:, col * BL:(col + 1) * BL],
                                     rhs=teT[:, hc * T:(hc + 1) * T],
                                     start=(hc == 0), stop=(hc == HC - 1))
            return f
        def soft_mk(ent):
            def f():
                sc = sc_t[ent]
                asm = const.tile([BL, T], F32, name=f"asm{ent}")
                ssum = const.tile([BL, 1], F32, name=f"ssum{ent}")
                nc.scalar.activation(asm[:], sc[:], AF.Exp, accum_out=ssum[:])
                rs = const.tile([BL, 1], F32, name=f"rs{ent}")
                nc.vector.reciprocal(rs[:], ssum[:])
                al = const.tile([BL, T], F32R, name=f"al{ent}")
                nc.vector.tensor_scalar(out=al[:], in0=asm[:], scalar1=rs[:, 0:1],
                                        scalar2=None, op0=ALU.mult)
                al_t[ent] = al
            return f
        def alt_mk(ent):
            def f():
                pt = ps_sm.tile([128, 128], F32R, tag="sm", name=f"alT{ent}")
                nc.tensor.transpose(pt[0:T, 0:BL], al_t[ent][:], ident[0:BL, 0:BL])
                alTe = const.tile([T, BL], BF16, name=f"alTe{ent}")
                nc.vector.tensor_copy(out=alTe[:], in_=pt[0:T, 0:BL].bitcast(F32))
                alTe_t[ent] = alTe
            return f
        def pet_mk(ent, hc0):
            def f():
                base = 16 if ent == 0 else 24
                for hc in (hc0, hc0 + 1):
                    pe = ps_sm.tile([128, BL], F32, tag="sm", name=f"pet{ent}{hc}")
                    nc.tensor.matmul(pe[:], lhsT=te_sb[:, hc * 128:(hc + 1) * 128],
                                     rhs=alTe_t[ent][:], start=True, stop=True)
                    nc.vector.tensor_copy(
                        out=efT[:, (base + hc) * BL:(base + hc + 1) * BL], in_=pe[:])
            return f
        for ent in range(2):
            for hc0 in range(0, HC, 2):
                steps.append((E_TYPE, sc_mk(ent, hc0)))
            steps.append((E_TYPE, soft_mk(ent)))
            steps.append((E_TYPE, alt_mk(ent)))
            for hc0 in range(0, HC, 2):
                steps.append((E_TYPE, pet_mk(ent, hc0)))

    def queue_bias_copy():
        def f():
            nc.vector.tensor_copy(out=bias_sb[:], in_=deT[:])
        steps.append((E_DE, f))

    def emit_tanh_chain(j):
        b, h = j // 2, j % 2
        u = work.tile([128, A], BF16, tag="u", name=f"u{j}")
        nc.scalar.activation(u[:, 0:256], dp_t[j][:, 0:256], AF.Tanh,
                             bias=bias_sb[:, (2 * h) * BL + b:(2 * h) * BL + b + 1])
        nc.scalar.activation(u[:, 256:512], dp_t[j][:, 256:512], AF.Tanh,
                             bias=bias_sb[:, (2 * h + 1) * BL + b:(2 * h + 1) * BL + b + 1])
        u_t[j] = u
        scr = work.tile([128, A], BF16, tag="scr", name=f"scr{j}")
        nc.vector.tensor_tensor(out=scr[:], in0=u[:], in1=v_bc[:], op=ALU.mult)
        nc.vector.tensor_reduce(out=vu[:, j:j + 1], in_=scr[:],
                                axis=mybir.AxisListType.X, op=ALU.add)

    def emit_z(j):
        # one PSUM group for the whole acc bank: start only on the very
        # first matmul of the drain; pending-zero covers every region
        b = j // 2
        nc.tensor.matmul(esum_row[0:1, b:b + 1], lhsT=expb[:, j:j + 1],
                         rhs=ones_bf[:], start=(j == 0), stop=False,
                         skip_group_check=True)
        for s in range(HC):
            nc.tensor.matmul(zt[:, s * BL + b:s * BL + b + 1],
                             lhsT=whz_v[:, j, s * 128:(s + 1) * 128],
                             rhs=expb[:, j:j + 1],
                             start=False,
                             stop=(j == NT - 1 and s == HC - 1),
                             skip_group_check=True)

    # queue all side work (PE bits chopped <=2 instrs so the 4-deep
    # wait-queue never clogs; deps gate execution)
    sc_t = [None, None]
    al_t = [None, None]
    alTe_t = [None, None]
    sc_t[0] = ps_sm.tile([BL, T], F32, tag="sm", name="sc0")
    sc_t[1] = ps_sm.tile([BL, T], F32, tag="sm", name="sc1")
    queue_entity_gather()
    queue_type_chain()
    queue_de(KE1, 0, went1_v, E_GATHER)
    queue_de(KE2, KE1, went2_v, E_DE)
    queue_bias_copy()

    sp = [0]

    def drain(i):
        n = 0
        while sp[0] < len(steps) and steps[sp[0]][0] <= i and n < 2:
            steps[sp[0]][1]()
            sp[0] += 1
            n += 1

    done_tanh = 0
    done_exp = 0

    def pump_exp():
        nonlocal done_exp
        while done_exp + 2 <= done_tanh - 2:
            j = done_exp
            nc.scalar.activation(expb[:, j:j + 2], vu[:, j:j + 2], AF.Exp)
            done_exp += 2

    for i in range(NT):
        emit_dp(i, drain)
        if i > E_DE + 1:
            while done_tanh <= i - 1:
                emit_tanh_chain(done_tanh)
                done_tanh += 1
                pump_exp()
    while sp[0] < len(steps):
        steps[sp[0]][1]()
        sp[0] += 1
    while done_tanh < NT:
        emit_tanh_chain(done_tanh)
        done_tanh += 1
        pump_exp()
    while done_exp < NT:
        j = done_exp
        nc.scalar.activation(expb[:, j:j + 2], vu[:, j:j + 2], AF.Exp)
        done_exp += 2

    # ---------- z drain + split epilogue (batches 0-5 stored early) ----------
    rec_row = const.tile([1, BL], F32, name="rec_row")
    rec_bc = const.tile([128, BL], F32, name="rec_bc")
    z_sb = const.tile([128, HC * BL], F32, name="z_sb")
    out_v = out_d.rearrange("(s p) b -> p s b", p=128)

    def emit_store(b0, b1):
        nb = b1 - b0
        nc.vector.reciprocal(rec_row[:, b0:b1], esum_row[:, b0:b1])
        nc.gpsimd.partition_broadcast(rec_bc[:, b0:b1], rec_row[0:1, b0:b1])
        rb_v = rec_bc[:, b0:b1].rearrange(
            "p (s b) -> p s b", s=1).broadcast_to([128, HC, nb])
        zt_v = zt.rearrange("p (s b) -> p s b", s=HC)[:, :, b0:b1]
        zs_v = z_sb.rearrange("p (s b) -> p s b", s=HC)[:, :, b0:b1]
        nc.vector.tensor_tensor(out=zs_v, in0=zt_v, in1=rb_v, op=ALU.mult)
        nc.sync.dma_start(out_v[:, :, b0:b1], zs_v)

    if _os.environ.get("ONESTORE"):
        for j in range(NT):
            emit_z(j)
        emit_store(0, BL)
    else:
        for j in range(12):
            emit_z(j)
        emit_store(0, 6)
        for j in range(12, NT):
            emit_z(j)
        emit_store(6, BL)

    if _os.environ.get("DBG"):
        dbg_bias = nc.dram_tensor("dbg_bias", [128, 32], F32, kind="ExternalOutput").ap()
        nc.sync.dma_start(dbg_bias[:], bias_sb[:])
        dbg_vu = nc.dram_tensor("dbg_vu", [128, NT], F32, kind="ExternalOutput").ap()
        nc.sync.dma_start(dbg_vu[:], vu[:])
        dbg_dp = nc.dram_tensor("dbg_dp", [128, A], F32, kind="ExternalOutput").ap()
        nc.sync.dma_start(dbg_dp[:], dp_t[0][:])
        dbg_ef = nc.dram_tensor("dbg_ef", [128, 256], F32, kind="ExternalOutput").ap()
        efc = const.tile([128, 256], F32, name="efc")
        nc.vector.tensor_copy(out=efc[:], in_=efT[:])
        nc.sync.dma_start(dbg_ef[:], efc[:])
    for p in (ps_acc, ps_sm, ps_dp, work, const):
        p.release()


def build():
    nc = bacc.Bacc("TRN2", target_bir_lowering=False, debug=False,
                   num_devices=NCORES)
    with tile.TileContext(nc) as tc:
        _build_core(tc)
    nc.compile()
    return nc


_NC = None


def _colperm():
    j = np.arange(A)
    s, p = j // 128, j % 128
    return 256 * (s // 2) + 2 * p + (s % 2)


def kernel(word_hiddens, pos_e1_embeddings, pos_e2_embeddings, e1_end, e2_end,
           type_embeddings, W_pos, W_ent, v):
    global _NC
    if _NC is None:
        _NC = build()
    BF = ml_dtypes.bfloat16
    F8 = ml_dtypes.float8_e4m3
    wh = np.ascontiguousarray(word_hiddens, dtype=np.float32).reshape(B, L, H)
    p1 = np.ascontiguousarray(pos_e1_embeddings, dtype=np.float32).reshape(B, L, P2)
    p2 = np.ascontiguousarray(pos_e2_embeddings, dtype=np.float32).reshape(B, L, P2)
    e1 = np.asarray(e1_end, dtype=np.int32).reshape(B)
    e2 = np.asarray(e2_end, dtype=np.int32).reshape(B)
    te = np.ascontiguousarray(type_embeddings, dtype=np.float32).astype(BF)
    perm = _colperm()
    we0 = np.asarray(W_ent, dtype=np.float32).reshape(4, H, A)
    went1 = np.concatenate([we0[0], we0[2]], axis=0)[:, perm]
    went2 = np.concatenate([we0[1], we0[3]], axis=0)[:, perm]
    went1 = np.ascontiguousarray(
        went1.reshape(KE1, 128, A).transpose(1, 0, 2).reshape(128, KE1 * A)).astype(BF)
    went2 = np.ascontiguousarray(
        went2.reshape(KE2, 128, A).transpose(1, 0, 2).reshape(128, KE2 * A)).astype(F8)
    wp = np.asarray(W_pos, dtype=np.float32)
    wpk = wp.reshape(KF, 128, A)
    wpos = np.ascontiguousarray(
        wpk[:KB].transpose(1, 0, 2).reshape(128, KB * A)).astype(BF)
    wpos8 = np.ascontiguousarray(
        wpk[KB:].transpose(1, 0, 2).reshape(128, 2 * A)).astype(F8)
    vv = np.ascontiguousarray(v, dtype=np.float32).reshape(1, A).astype(BF)

    in_maps = []
    for c in range(NCORES):
        s = slice(c * BL, (c + 1) * BL)
        whc = np.ascontiguousarray(wh[s].reshape(TOK, H))
        pf = np.empty((TOK, F), dtype=np.float32)
        pf[:, :H] = whc
        pf[:, H:H + P2] = p1[s].reshape(TOK, P2)
        pf[:, H + P2:] = p2[s].reshape(TOK, P2)
        pfk = pf.reshape(NT, 128, KF, 128)
        whT = np.ascontiguousarray(
            pfk[:, :, :KB].transpose(3, 0, 2, 1)
            .reshape(128, NT * KB * 128)).astype(BF)
        whT8 = np.ascontiguousarray(
            pfk[:, :, KB:].transpose(3, 0, 2, 1)
            .reshape(128, NT * 2 * 128)).astype(F8)
        whz = np.ascontiguousarray(
            whc.reshape(NT, 128, H).transpose(1, 0, 2).reshape(128, NT * H)).astype(BF)
        ends = np.concatenate([e1[s], e2[s]]).reshape(2 * BL, 1)
        in_maps.append({
            "whT": whT,
            "whT8": whT8,
            "W_pos8": wpos8,
            "whz": whz,
            "W_pos": wpos,
            "went1": went1,
            "went2": went2,
            "word_hiddens": whc,
            "ends": np.ascontiguousarray(ends),
            "type_embeddings": te,
            "v": vv,
        })
    res = run_bass_kernel_spmd(_NC, in_maps, core_ids=list(range(NCORES)))
    return np.concatenate(
        [res.results[c]["out"].T for c in range(NCORES)], axis=0).astype(np.float32)


# revision 17
# speedup vs baseline: 1.5060x; 1.0252x over previous
"""EntityAwareAttention TRN2 Bass kernel — 8-core data parallel, v2.

Cost-model-driven redesign vs baseline:
  * dense_ent computed TRANSPOSED (deT[a_perm, b]): lhsT = W_ent' k-tile
    (stationary, free), rhs = efT [128, 8] -> out [128, 8] = 8 cycles/instr
    instead of out [8, 512] = 512 cycles. W_ent columns are host-permuted so
    deT slices ARE the per-partition tanh-bias columns directly (one copy).
  * z computed TRANSPOSED (zT[h, b]): lhsT = wh tile h-slice (stationary),
    rhs = exp column [128, 1] -> out [128, 1] = 1 cycle/instr, accumulated
    per batch across its two token tiles. Replaces 16.4K PE cycles.
  * vu via ONE fused DVE tensor_tensor_reduce per tile (bf16 operands).
  * exp kept unnormalized; esum via two strided-column ones-matmuls.
  * PE "heater": dummy matmuls at the head keep the PE p-state ramp warm
    while the first DMAs land (cold PE runs at 1.2GHz for 3us).
  * W2/W4 (the e*_type halves of W_ent) shipped fp8-e4m3 (error-neutral:
    type features are ~3% of dense_ent magnitude).
  * All weights resident in SBUF (no buffer recycling); DMAs are few, large,
    partition-major-contiguous, and ordered so the PE pipeline never waits.

Numerics: rel-err budget is 2e-2; this design measures ~2.6e-3 in numpy.
"""

import numpy as np
import ml_dtypes

import concourse.bass as bass
import concourse.tile as tile
from concourse import bacc, mybir
from concourse.bass_utils import run_bass_kernel_spmd

F32 = mybir.dt.float32
F32R = mybir.dt.float32r
BF16 = mybir.dt.bfloat16
FP8 = mybir.dt.float8e4
I32 = mybir.dt.int32
AF = mybir.ActivationFunctionType
ALU = mybir.AluOpType

B, L, H, P2, A, T = 64, 256, 1024, 64, 512, 8
NCORES = 8
BL = B // NCORES            # 8 local batches
TOK = BL * L                # 2048 tokens
NT = TOK // 128             # 16 token tiles
F = H + 2 * P2              # 1152 contraction dim
KF = F // 128               # 9 k-tiles for dense_pos
KB = 7                      # bf16 k-tiles; k7+k8 go fp8 DoubleRow
KE1 = 16                    # W1;W3 k-tiles (e_h halves)
KE2 = 16                    # W2;W4 k-tiles (type halves)
HC = H // 128               # 8 h-chunks

# ---- tuning knobs (sim-derived) ----
N_HEAT = 80                 # heater matmuls at the head
MID_HEAT = 75               # heater matmuls between dp0 k2 and k3 (wpos wait)
HEAT_COLS = 64
LAG = 8                     # tanh for tile j gated at dp of tile j+LAG
NCOPY = 8                   # dp tiles 0..NCOPY-1 copied PSUM->SBUF (bias wait)
import os as _os
E_GATHER = int(_os.environ.get("E_GATHER", 0))
E_TYPE = int(_os.environ.get("E_TYPE", 1))
E_DE = int(_os.environ.get("E_DE", 5))


def _build_core(tc):
    nc = tc.nc
    whT_d = nc.dram_tensor("whT", [128, NT * KB * 128], BF16, kind="ExternalInput").ap()
    whT8_d = nc.dram_tensor("whT8", [128, NT * 2 * 128], FP8, kind="ExternalInput").ap()
    whz_d = nc.dram_tensor("whz", [128, NT * H], BF16, kind="ExternalInput").ap()
    wpos_d = nc.dram_tensor("W_pos", [128, KB * A], BF16, kind="ExternalInput").ap()
    wpos8_d = nc.dram_tensor("W_pos8", [128, 2 * A], FP8, kind="ExternalInput").ap()
    went1_d = nc.dram_tensor("went1", [128, KE1 * A], BF16, kind="ExternalInput").ap()
    went2_d = nc.dram_tensor("went2", [128, KE2 * A], FP8, kind="ExternalInput").ap()
    wh_d = nc.dram_tensor("word_hiddens", [TOK, H], F32R, kind="ExternalInput").ap()
    ends_d = nc.dram_tensor("ends", [2 * BL, 1], I32, kind="ExternalInput").ap()
    te_d = nc.dram_tensor("type_embeddings", [T, H], BF16, kind="ExternalInput").ap()
    v_d = nc.dram_tensor("v", [1, A], BF16, kind="ExternalInput").ap()
    out_d = nc.dram_tensor("out", [HC * 128, BL], F32, kind="ExternalOutput").ap()

    const = tc.alloc_tile_pool(name="const", bufs=1)
    work = tc.alloc_tile_pool(name="work", bufs=3)
    ps_dp = tc.alloc_tile_pool(name="ps_dp", bufs=5, space="PSUM")
    ps_sm = tc.alloc_tile_pool(name="ps_sm", bufs=2, space="PSUM")
    ps_acc = tc.alloc_tile_pool(name="ps_acc", bufs=1, space="PSUM")

    # ---------- heater constants (Pool engine, ready ~0.5us) ----------
    hl = const.tile([128, HEAT_COLS], BF16, name="hl")
    nc.gpsimd.memset(hl[:], 0.0)

    # ---------- DMA schedule ----------
    # SP queue: the big weight stream, most-urgent first.
    whT = const.tile([128, NT * KB * 128], BF16, name="whT")
    whT8 = const.tile([128, NT * 2 * 128], FP8, name="whT8")
    wpos = const.tile([128, KB * A], BF16, name="wpos")
    wpos8 = const.tile([128, 2 * A], FP8, name="wpos8")
    whz = const.tile([128, NT * H], BF16, name="whz")
    went1 = const.tile([128, KE1 * A], BF16, name="went1")
    went2 = const.tile([128, KE2 * A], FP8, name="went2")

    CW = KB * 128  # whT bf16 columns per token tile

    def whT_dma(lo, hi):
        nc.sync.dma_start(whT[:, lo * CW:hi * CW], whT_d[:, lo * CW:hi * CW])

    # small loads first: tiny transfers, but the gather chain hangs off ends
    # (ACT-issued DMA faults the device, so everything goes via SP)
    ends = const.tile([2 * BL, 1], I32, name="ends")
    nc.sync.dma_start(ends[:], ends_d[:])
    te_sb = const.tile([T, H], BF16, name="te_sb")
    nc.sync.dma_start(te_sb[:], te_d[:])
    v_sb = const.tile([1, A], BF16, name="v_sb")
    nc.sync.dma_start(v_sb[:], v_d[:])
    nc.sync.dma_start(wpos[:, 0:3 * A], wpos_d[:, 0:3 * A])              # k0-2
    whT_dma(0, 1)
    nc.sync.dma_start(wpos[:, 3 * A:], wpos_d[:, 3 * A:])                # k3-6
    nc.sync.dma_start(wpos8[:], wpos8_d[:])
    HT8 = NT * 128  # half of whT8 columns
    nc.sync.dma_start(whT8[:, 0:HT8], whT8_d[:, 0:HT8])
    whT_dma(1, 2)
    nc.sync.dma_start(whT8[:, HT8:], whT8_d[:, HT8:])
    whT_dma(2, 3)
    whT_dma(3, 4)
    import os
    _ORD = os.environ.get("DMA_ORD", "F")
    went_dmas = [
        lambda: nc.sync.dma_start(went1[:, 0:8 * A], went1_d[:, 0:8 * A]),
        lambda: nc.sync.dma_start(went1[:, 8 * A:], went1_d[:, 8 * A:]),
        lambda: nc.sync.dma_start(went2[:], went2_d[:]),
    ]
    whz_dmas = [
        lambda: nc.sync.dma_start(whz[:, 0:4 * H], whz_d[:, 0:4 * H]),
        lambda: nc.sync.dma_start(whz[:, 4 * H:8 * H], whz_d[:, 4 * H:8 * H]),
        lambda: nc.sync.dma_start(whz[:, 8 * H:12 * H], whz_d[:, 8 * H:12 * H]),
        lambda: nc.sync.dma_start(whz[:, 12 * H:], whz_d[:, 12 * H:]),
    ]
    whT_chunks = [lambda lo=lo: whT_dma(lo, lo + 2) for lo in range(4, 16, 2)]
    # each config: list of ('T', i) / ('E', i) / ('Z', i)
    ORDS = {
        # went all first (prev)
        "A": ["E0", "E1", "E2", "T0", "T1", "T2", "T3", "T4", "T5",
               "Z0", "Z1", "Z2", "Z3"],
        # went interleaved every other whT chunk
        "B": ["E0", "T0", "E1", "T1", "E2", "T2", "T3", "T4", "T5",
               "Z0", "Z1", "Z2", "Z3"],
        # went slightly later
        "C": ["T0", "E0", "T1", "E1", "T2", "E2", "T3", "T4", "T5",
               "Z0", "Z1", "Z2", "Z3"],
        # went2 first (group flip not needed: de_ty start flag handles)
        "D": ["E2", "T0", "E0", "T1", "E1", "T2", "T3", "T4", "T5",
               "Z0", "Z1", "Z2", "Z3"],
        # whz earlier, went mid
        "E": ["E0", "T0", "E1", "T1", "E2", "T2", "T3", "Z0", "T4",
               "Z1", "T5", "Z2", "Z3"],
        # big whT head: tiles 4-7 before went; rest JIT after
        "F": ["T0", "T1", "E0", "E1", "E2", "T2", "T3", "T4", "T5",
               "Z0", "Z1", "Z2", "Z3"],
        # same + whz interleaved among tail whT
        "G": ["T0", "T1", "E0", "E1", "E2", "T2", "Z0", "T3", "Z1",
               "T4", "Z2", "T5", "Z3"],
        # even bigger head
        "H": ["T0", "T1", "T2", "E0", "E1", "E2", "T3", "T4", "T5",
               "Z0", "Z1", "Z2", "Z3"],
        # went interleaved between whT chunks
        "I": ["T0", "T1", "E0", "T2", "E1", "T3", "E2", "T4", "T5",
               "Z0", "Z1", "Z2", "Z3"],
    }
    for tok in ORDS[_ORD]:
        kind, idx = tok[0], int(tok[1:])
        if kind == "T":
            whT_chunks[idx]()
        elif kind == "E":
            went_dmas[idx]()
        else:
            whz_dmas[idx]()



    # ---------- gather chain (gpsimd/DVE; latency-critical) ----------
    gidx = const.tile([2 * BL, 1], I32, name="gidx")
    nc.gpsimd.iota(gidx[:], pattern=[[1, 1]], base=0, channel_multiplier=L)
    nc.vector.tensor_scalar(out=gidx[:], in0=gidx[:], scalar1=BL * L - 1,
                            scalar2=None, op0=ALU.bitwise_and)
    nc.vector.tensor_tensor(out=gidx[:], in0=gidx[:], in1=ends[:], op=ALU.add)
    eh = const.tile([2 * BL, H], F32R, name="eh")
    nc.gpsimd.indirect_dma_start(
        out=eh[:], out_offset=None, in_=wh_d[:],
        in_offset=bass.IndirectOffsetOnAxis(ap=gidx[:, 0:1], axis=0))

    # ---------- other small device constants ----------
    iota_p = const.tile([128, 128], I32, name="iota_p")
    iota_f = const.tile([128, 128], I32, name="iota_f")
    nc.gpsimd.iota(iota_p[:], pattern=[[0, 128]], base=0, channel_multiplier=1)
    nc.gpsimd.iota(iota_f[:], pattern=[[1, 128]], base=0, channel_multiplier=0)
    ident = const.tile([128, 128], F32R, name="ident")
    nc.vector.tensor_tensor(out=ident[:], in0=iota_p[:], in1=iota_f[:],
                            op=ALU.is_equal)
    ident_bf = const.tile([128, 128], BF16, name="ident_bf")
    nc.vector.tensor_copy(out=ident_bf[:], in_=ident[:].bitcast(F32))
    ones_bf = const.tile([128, 1], BF16, name="ones_bf")
    nc.gpsimd.memset(ones_bf[:], 1.0)
    v_bc = const.tile([128, A], BF16, name="v_bc")
    nc.gpsimd.partition_broadcast(v_bc[:], v_sb[0:1, :])

    # ---------- PE program ----------
    # 0) heater: keep PE busy (and ramping) until real operands land.
    heat_ps = ps_sm.tile([128, HEAT_COLS], F32, name="heat_ps", tag="sm")
    for _ in range(N_HEAT):
        nc.tensor.matmul(heat_ps[0:HEAT_COLS, :], lhsT=hl[:], rhs=hl[:],
                         start=True, stop=True, skip_group_check=True)

    whT_v = whT.rearrange("p (i k c) -> p i k c", i=NT, k=KB)
    whT8_v = whT8.rearrange("p (i two c) -> p i two c", i=NT, two=2)
    wpos8_v = wpos8.rearrange("p (two a) -> p two a", two=2)
    wpos_v = wpos.rearrange("p (k a) -> p k a", k=KB)
    whz_v = whz.rearrange("p (i h) -> p i h", i=NT)
    went1_v = went1.rearrange("p (k a) -> p k a", k=KE1)
    went2_v = went2.rearrange("p (k a) -> p k a", k=KE2)

    teT = const.tile([128, HC * T], BF16, name="teT")
    efT = const.tile([128, 32 * BL], BF16, name="efT")
    acc = ps_acc.tile([128, 128], F32, name="acc", tag="acc")
    deT = acc[:, 0:4 * BL]
    zt = acc[:, 32:32 + HC * BL]
    esum_row = acc[0:1, 96:96 + BL]
    bias_sb = const.tile([128, 4 * BL], F32, name="bias_sb")
    vu = const.tile([128, NT], F32, name="vu")
    expb = const.tile([128, NT], BF16, name="expb")
    u_t = [None] * NT
    dp_t = [None] * NT

    steps = []          # (min_tile, thunk) — drained between dp k-matmuls

    def emit_dp(i, drain):
        dp = ps_dp.tile([128, A], F32, tag="dp", name=f"dp{i}")
        for k in range(KB):
            nc.tensor.matmul(dp[:], lhsT=whT_v[:, i, k, :], rhs=wpos_v[:, k, :],
                             start=(k == 0), stop=False)
            if i == 0 and k == 2:
                for _ in range(MID_HEAT):
                    nc.tensor.matmul(heat_ps[0:HEAT_COLS, :], lhsT=hl[:],
                                     rhs=hl[:], start=True, stop=True,
                                     skip_group_check=True)
            drain(i)
        if _os.environ.get("NODR"):
            nc.tensor.matmul(dp[:], lhsT=whT8_v[:, i, 0, :], rhs=wpos8_v[:, 0, :],
                             start=False, stop=False)
            nc.tensor.matmul(dp[:], lhsT=whT8_v[:, i, 1, :], rhs=wpos8_v[:, 1, :],
                             start=False, stop=True)
        else:
            nc.tensor.matmul(dp[:], lhsT=whT8_v[:, i, :, :], rhs=wpos8_v[:],
                             start=False, stop=True,
                             perf_mode=mybir.MatmulPerfMode.DoubleRow)
        drain(i)
        if i < NCOPY:
            dpc = const.tile([128, A], F32, name=f"dpc{i}")
            if i % 2 == 0:
                nc.vector.tensor_copy(out=dpc[:], in_=dp[:])
            else:
                nc.scalar.copy(dpc[:], dp[:])
            dp_t[i] = dpc
        else:
            dp_t[i] = dp

    def queue_entity_gather():
        def tr(hc):
            def f():
                pt = ps_sm.tile([128, 128], F32R, tag="sm", name=f"ehT{hc}")
                nc.tensor.transpose(pt[:, 0:2 * BL], eh[:, hc * 128:(hc + 1) * 128],
                                    ident[0:2 * BL, 0:2 * BL])
                nc.vector.tensor_copy(out=efT[:, hc * BL:(hc + 1) * BL],
                                      in_=pt[:, 0:BL].bitcast(F32))
                nc.vector.tensor_copy(out=efT[:, (HC + hc) * BL:(HC + hc + 1) * BL],
                                      in_=pt[:, BL:2 * BL].bitcast(F32))
            return f
        def trte(hc):
            def f():
                pt = ps_sm.tile([128, 128], F32R, tag="sm", name=f"teT{hc}")
                ptb = pt.bitcast(BF16)
                nc.tensor.transpose(ptb[:, 0:T], te_sb[:, hc * 128:(hc + 1) * 128],
                                    ident_bf[0:T, 0:T])
                nc.vector.tensor_copy(out=teT[:, hc * T:(hc + 1) * T],
                                      in_=ptb[:, 0:T])
            return f
        for hc in range(HC):
            steps.append((E_GATHER, tr(hc)))
        for hc in range(HC):
            steps.append((E_GATHER, trte(hc)))

    def queue_de(kts, base, w_v, min_tile):
        # dense_ent k-tiles: 2 matmuls per step; every column region resets
        # on the overall first k-tile and closes on the overall last one.
        def mk(kt, s0):
            def f():
                for s in (s0, s0 + 1):
                    nc.tensor.matmul(deT[:, s * BL:(s + 1) * BL],
                                     lhsT=w_v[:, kt, s * 128:(s + 1) * 128],
                                     rhs=efT[:, (base + kt) * BL:(base + kt + 1) * BL],
                                     start=(base + kt == 0 and s == 0),
                                     stop=(base + kt == KE1 + KE2 - 1 and s == 3),
                                     skip_group_check=True)
            return f
        for kt in range(kts):
            steps.append((min_tile, mk(kt, 0)))
            steps.append((min_tile, mk(kt, 2)))

    def queue_type_chain():
        def sc_mk(ent, hc0):
            def f():
                sc = sc_t[ent]
                for hc in (hc0, hc0 + 1):
                    col = (0 if ent == 0 else HC) + hc
                    nc.tensor.matmul(sc[:], lhsT=efT[:, col * BL:(col + 1) * BL],
                                     rhs=teT[:, hc * T:(hc + 1) * T],
                                     start=(hc == 0), stop=(hc == HC - 1))
            return f
        def soft_mk(ent):
            def f():
                sc = sc_t[ent]
                asm = const.tile([BL, T], F32, name=f"asm{ent}")
                ssum = const.tile([BL, 1], F32, name=f"ssum{ent}")
                nc.scalar.activation(asm[:], sc[:], AF.Exp, accum_out=ssum[:])
                rs = const.tile([BL, 1], F32, name=f"rs{ent}")
                nc.vector.reciprocal(rs[:], ssum[:])
                al = const.tile([BL, T], F32R, name=f"al{ent}")
                nc.vector.tensor_scalar(out=al[:], in0=asm[:], scalar1=rs[:, 0:1],
                                        scalar2=None, op0=ALU.mult)
                al_t[ent] = al
            return f
        def alt_mk(ent):
            def f():
                pt = ps_sm.tile([128, 128], F32R, tag="sm", name=f"alT{ent}")
                nc.tensor.transpose(pt[0:T, 0:BL], al_t[ent][:], ident[0:BL, 0:BL])
                alTe = const.tile([T, BL], BF16, name=f"alTe{ent}")
                nc.vector.tensor_copy(out=alTe[:], in_=pt[0:T, 0:BL].bitcast(F32))
                alTe_t[ent] = alTe
            return f
        def pet_mk(ent, hc0):
            def f():
                base = 16 if ent == 0 else 24
                for hc in (hc0, hc0 + 1):
                    pe = ps_sm.tile([128, BL], F32, tag="sm", name=f"pet{ent}{hc}")
                    nc.tensor.matmul(pe[:], lhsT=te_sb[:, hc * 128:(hc + 1) * 128],
                                     rhs=alTe_t[ent][:], start=True, stop=True)
                    nc.vector.tensor_copy(
                        out=efT[:, (base + hc) * BL:(base + hc + 1) * BL], in_=pe[:])
            return f
        for ent in range(2):
            for hc0 in range(0, HC, 2):
                steps.append((E_TYPE, sc_mk(ent, hc0)))
            steps.append((E_TYPE, soft_mk(ent)))
            steps.append((E_TYPE, alt_mk(ent)))
            for hc0 in range(0, HC, 2):
                steps.append((E_TYPE, pet_mk(ent, hc0)))

    def queue_bias_copy():
        def f():
            nc.vector.tensor_copy(out=bias_sb[:], in_=deT[:])
        steps.append((E_DE, f))

    def emit_tanh_chain(j):
        b, h = j // 2, j % 2
        u = work.tile([128, A], BF16, tag="u", name=f"u{j}")
        nc.scalar.activation(u[:, 0:256], dp_t[j][:, 0:256], AF.Tanh,
                             bias=bias_sb[:, (2 * h) * BL + b:(2 * h) * BL + b + 1])
        nc.scalar.activation(u[:, 256:512], dp_t[j][:, 256:512], AF.Tanh,
                             bias=bias_sb[:, (2 * h + 1) * BL + b:(2 * h + 1) * BL + b + 1])
        u_t[j] = u
        scr = work.tile([128, A], BF16, tag="scr", name=f"scr{j}")
        nc.vector.tensor_tensor(out=scr[:], in0=u[:], in1=v_bc[:], op=ALU.mult)
        nc.vector.tensor_reduce(out=vu[:, j:j + 1], in_=scr[:],
                                axis=mybir.AxisListType.X, op=ALU.add)

    def emit_z(j):
        # one PSUM group for the whole acc bank: start only on the very
        # first matmul of the drain; pending-zero covers every region
        b = j // 2
        nc.tensor.matmul(esum_row[0:1, b:b + 1], lhsT=expb[:, j:j + 1],
                         rhs=ones_bf[:], start=(j == 0), stop=False,
                         skip_group_check=True)
        for s in range(HC):
            nc.tensor.matmul(zt[:, s * BL + b:s * BL + b + 1],
                             lhsT=whz_v[:, j, s * 128:(s + 1) * 128],
                             rhs=expb[:, j:j + 1],
                             start=False,
                             stop=(j == NT - 1 and s == HC - 1),
                             skip_group_check=True)

    # queue all side work (PE bits chopped <=2 instrs so the 4-deep
    # wait-queue never clogs; deps gate execution)
    sc_t = [None, None]
    al_t = [None, None]
    alTe_t = [None, None]
    sc_t[0] = ps_sm.tile([BL, T], F32, tag="sm", name="sc0")
    sc_t[1] = ps_sm.tile([BL, T], F32, tag="sm", name="sc1")
    queue_entity_gather()
    queue_type_chain()
    queue_de(KE1, 0, went1_v, E_GATHER)
    queue_de(KE2, KE1, went2_v, E_DE)
    queue_bias_copy()

    sp = [0]

    def drain(i):
        n = 0
        while sp[0] < len(steps) and steps[sp[0]][0] <= i and n < 2:
            steps[sp[0]][1]()
            sp[0] += 1
            n += 1

    done_tanh = 0
    done_exp = 0

    def pump_exp():
        nonlocal done_exp
        while done_exp + 2 <= done_tanh - 2:
            j = done_exp
            nc.scalar.activation(expb[:, j:j + 2], vu[:, j:j + 2], AF.Exp)
            done_exp += 2

    for i in range(NT):
        emit_dp(i, drain)
        if i > E_DE + 1:
            while done_tanh <= i - 1:
                emit_tanh_chain(done_tanh)
                done_tanh += 1
                pump_exp()
    while sp[0] < len(steps):
        steps[sp[0]][1]()
        sp[0] += 1
    while done_tanh < NT:
        emit_tanh_chain(done_tanh)
        done_tanh += 1
        pump_exp()
    while done_exp < NT:
        j = done_exp
        nc.scalar.activation(expb[:, j:j + 2], vu[:, j:j + 2], AF.Exp)
        done_exp += 2

    # ---------- z drain + split epilogue (batches 0-5 stored early) ----------
    rec_row = const.tile([1, BL], F32, name="rec_row")
    rec_bc = const.tile([128, BL], F32, name="rec_bc")
    z_sb = const.tile([128, HC * BL], F32, name="z_sb")
    out_v = out_d.rearrange("(s p) b -> p s b", p=128)

    def emit_store(b0, b1):
        nb = b1 - b0
        nc.vector.reciprocal(rec_row[:, b0:b1], esum_row[:, b0:b1])
        nc.gpsimd.partition_broadcast(rec_bc[:, b0:b1], rec_row[0:1, b0:b1])
        rb_v = rec_bc[:, b0:b1].rearrange(
            "p (s b) -> p s b", s=1).broadcast_to([128, HC, nb])
        zt_v = zt.rearrange("p (s b) -> p s b", s=HC)[:, :, b0:b1]
        zs_v = z_sb.rearrange("p (s b) -> p s b", s=HC)[:, :, b0:b1]
        nc.vector.tensor_tensor(out=zs_v, in0=zt_v, in1=rb_v, op=ALU.mult)
        nc.sync.dma_start(out_v[:, :, b0:b1], zs_v)

    if _os.environ.get("ONESTORE"):
        for j in range(NT):
            emit_z(j)
        emit_store(0, BL)
    else:
        for j in range(12):
            emit_z(j)
        emit_store(0, 6)
        for j in range(12, NT):
            emit_z(j)
        emit_store(6, BL)

    if _os.environ.get("DBG"):
        dbg_bias = nc.dram_tensor("dbg_bias", [128, 32], F32, kind="ExternalOutput").ap()
        nc.sync.dma_start(dbg_bias[:], bias_sb[:])
        dbg_vu = nc.dram_tensor("dbg_vu", [128, NT], F32, kind="ExternalOutput").ap()
        nc.sync.dma_start(dbg_vu[:], vu[:])
        dbg_dp = nc.dram_tensor("dbg_dp", [128, A], F32, kind="ExternalOutput").ap()
        nc.sync.dma_start(dbg_dp[:], dp_t[0][:])
        dbg_ef = nc.dram_tensor("dbg_ef", [128, 256], F32, kind="ExternalOutput").ap()
        efc = const.tile([128, 256], F32, name="efc")
        nc.vector.tensor_copy(out=efc[:], in_=efT[:])
        nc.sync.dma_start(dbg_ef[:], efc[:])
    for p in (ps_acc, ps_sm, ps_dp, work, const):
        p.release()


def build():
    nc = bacc.Bacc("TRN2", target_bir_lowering=False, debug=False,
                   num_devices=NCORES)
    with tile.TileContext(nc) as tc:
        _build_core(tc)
    nc.compile()
    return nc


_NC = None


def _colperm():
    j = np.arange(A)
    s, p = j // 128, j % 128
    return 256 * (s // 2) + 2 * p + (s % 2)


def kernel(word_hiddens, pos_e1_embeddings, pos_e2_embeddings, e1_end, e2_end,
           type_embeddings, W_pos, W_ent, v):
    global _NC
    if _NC is None:
        _NC = build()
    BF = ml_dtypes.bfloat16
    F8 = ml_dtypes.float8_e4m3
    wh = np.ascontiguousarray(word_hiddens, dtype=np.float32).reshape(B, L, H)
    p1 = np.ascontiguousarray(pos_e1_embeddings, dtype=np.float32).reshape(B, L, P2)
    p2 = np.ascontiguousarray(pos_e2_embeddings, dtype=np.float32).reshape(B, L, P2)
    e1 = np.asarray(e1_end, dtype=np.int32).reshape(B)
    e2 = np.asarray(e2_end, dtype=np.int32).reshape(B)
    te = np.ascontiguousarray(type_embeddings, dtype=np.float32).astype(BF)
    perm = _colperm()
    we0 = np.asarray(W_ent, dtype=np.float32).reshape(4, H, A)
    went1 = np.concatenate([we0[0], we0[2]], axis=0)[:, perm]
    went2 = np.concatenate([we0[1], we0[3]], axis=0)[:, perm]
    went1 = np.ascontiguousarray(
        went1.reshape(KE1, 128, A).transpose(1, 0, 2).reshape(128, KE1 * A)).astype(BF)
    went2 = np.ascontiguousarray(
        went2.reshape(KE2, 128, A).transpose(1, 0, 2).reshape(128, KE2 * A)).astype(F8)
    wp = np.asarray(W_pos, dtype=np.float32)
    wpk = wp.reshape(KF, 128, A)
    wpos = np.ascontiguousarray(
        wpk[:KB].transpose(1, 0, 2).reshape(128, KB * A)).astype(BF)
    wpos8 = np.ascontiguousarray(
        wpk[KB:].transpose(1, 0, 2).reshape(128, 2 * A)).astype(F8)
    vv = np.ascontiguousarray(v, dtype=np.float32).reshape(1, A).astype(BF)

    in_maps = []
    for c in range(NCORES):
        s = slice(c * BL, (c + 1) * BL)
        whc = np.ascontiguousarray(wh[s].reshape(TOK, H))
        pf = np.empty((TOK, F), dtype=np.float32)
        pf[:, :H] = whc
        pf[:, H:H + P2] = p1[s].reshape(TOK, P2)
        pf[:, H + P2:] = p2[s].reshape(TOK, P2)
        pfk = pf.reshape(NT, 128, KF, 128)
        whT = np.ascontiguousarray(
            pfk[:, :, :KB].transpose(3, 0, 2, 1)
            .reshape(128, NT * KB * 128)).astype(BF)
        whT8 = np.ascontiguousarray(
            pfk[:, :, KB:].transpose(3, 0, 2, 1)
            .reshape(128, NT * 2 * 128)).astype(F8)
        whz = np.ascontiguousarray(
            whc.reshape(NT, 128, H).transpose(1, 0, 2).reshape(128, NT * H)).astype(BF)
        ends = np.concatenate([e1[s], e2[s]]).reshape(2 * BL, 1)
        in_maps.append({
            "whT": whT,
            "whT8": whT8,
            "W_pos8": wpos8,
            "whz": whz,
            "W_pos": wpos,
            "went1": went1,
            "went2": went2,
            "word_hiddens": whc,
            "ends": np.ascontiguousarray(ends),
            "type_embeddings": te,
            "v": vv,
        })
    res = run_bass_kernel_spmd(_NC, in_maps, core_ids=list(range(NCORES)))
    return np.concatenate(
        [res.results[c]["out"].T for c in range(NCORES)], axis=0).astype(np.float32)


# revision 19
# speedup vs baseline: 1.5077x; 1.0011x over previous
"""EntityAwareAttention TRN2 Bass kernel — 8-core data parallel, v2.

Cost-model-driven redesign vs baseline:
  * dense_ent computed TRANSPOSED (deT[a_perm, b]): lhsT = W_ent' k-tile
    (stationary, free), rhs = efT [128, 8] -> out [128, 8] = 8 cycles/instr
    instead of out [8, 512] = 512 cycles. W_ent columns are host-permuted so
    deT slices ARE the per-partition tanh-bias columns directly (one copy).
  * z computed TRANSPOSED (zT[h, b]): lhsT = wh tile h-slice (stationary),
    rhs = exp column [128, 1] -> out [128, 1] = 1 cycle/instr, accumulated
    per batch across its two token tiles. Replaces 16.4K PE cycles.
  * vu via ONE fused DVE tensor_tensor_reduce per tile (bf16 operands).
  * exp kept unnormalized; esum via two strided-column ones-matmuls.
  * PE "heater": dummy matmuls at the head keep the PE p-state ramp warm
    while the first DMAs land (cold PE runs at 1.2GHz for 3us).
  * W2/W4 (the e*_type halves of W_ent) shipped fp8-e4m3 (error-neutral:
    type features are ~3% of dense_ent magnitude).
  * All weights resident in SBUF (no buffer recycling); DMAs are few, large,
    partition-major-contiguous, and ordered so the PE pipeline never waits.

Numerics: rel-err budget is 2e-2; this design measures ~2.6e-3 in numpy.
"""

import numpy as np
import ml_dtypes

import concourse.bass as bass
import concourse.tile as tile
from concourse import bacc, mybir
from concourse.bass_utils import run_bass_kernel_spmd

F32 = mybir.dt.float32
F32R = mybir.dt.float32r
BF16 = mybir.dt.bfloat16
FP8 = mybir.dt.float8e4
I32 = mybir.dt.int32
AF = mybir.ActivationFunctionType
ALU = mybir.AluOpType

B, L, H, P2, A, T = 64, 256, 1024, 64, 512, 8
NCORES = 8
BL = B // NCORES            # 8 local batches
TOK = BL * L                # 2048 tokens
NT = TOK // 128             # 16 token tiles
F = H + 2 * P2              # 1152 contraction dim
KF = F // 128               # 9 k-tiles for dense_pos
KB = 7                      # bf16 k-tiles; k7+k8 go fp8 DoubleRow
KE1 = 16                    # W1;W3 k-tiles (e_h halves)
KE2 = 16                    # W2;W4 k-tiles (type halves)
HC = H // 128               # 8 h-chunks

# ---- tuning knobs (sim-derived) ----
N_HEAT = 80                 # heater matmuls at the head
MID_HEAT = 92               # heater matmuls between dp0 k2 and k3 (wpos wait)
HEAT_COLS = 64
LAG = 8                     # tanh for tile j gated at dp of tile j+LAG
NCOPY = 8                   # dp tiles 0..NCOPY-1 copied PSUM->SBUF (bias wait)
import os as _os
E_GATHER = int(_os.environ.get("E_GATHER", 0))
E_TYPE = int(_os.environ.get("E_TYPE", 1))
E_DE = int(_os.environ.get("E_DE", 5))


def _build_core(tc):
    nc = tc.nc
    whT_d = nc.dram_tensor("whT", [128, NT * KB * 128], BF16, kind="ExternalInput").ap()
    whT8_d = nc.dram_tensor("whT8", [128, NT * 2 * 128], FP8, kind="ExternalInput").ap()
    whz_d = nc.dram_tensor("whz", [128, NT * H], BF16, kind="ExternalInput").ap()
    wpos_d = nc.dram_tensor("W_pos", [128, KB * A], BF16, kind="ExternalInput").ap()
    wpos8_d = nc.dram_tensor("W_pos8", [128, 2 * A], FP8, kind="ExternalInput").ap()
    went1_d = nc.dram_tensor("went1", [128, KE1 * A], BF16, kind="ExternalInput").ap()
    went2_d = nc.dram_tensor("went2", [128, KE2 * A], FP8, kind="ExternalInput").ap()
    wh_d = nc.dram_tensor("word_hiddens", [TOK, H], F32R, kind="ExternalInput").ap()
    ends_d = nc.dram_tensor("ends", [2 * BL, 1], I32, kind="ExternalInput").ap()
    te_d = nc.dram_tensor("type_embeddings", [T, H], BF16, kind="ExternalInput").ap()
    v_d = nc.dram_tensor("v", [1, A], BF16, kind="ExternalInput").ap()
    out_d = nc.dram_tensor("out", [HC * 128, BL], F32, kind="ExternalOutput").ap()

    const = tc.alloc_tile_pool(name="const", bufs=1)
    work = tc.alloc_tile_pool(name="work", bufs=3)
    ps_dp = tc.alloc_tile_pool(name="ps_dp", bufs=5, space="PSUM")
    ps_sm = tc.alloc_tile_pool(name="ps_sm", bufs=2, space="PSUM")
    ps_acc = tc.alloc_tile_pool(name="ps_acc", bufs=1, space="PSUM")

    # ---------- heater constants (Pool engine, ready ~0.5us) ----------
    hl = const.tile([128, HEAT_COLS], BF16, name="hl")
    nc.gpsimd.memset(hl[:], 0.0)

    # ---------- DMA schedule ----------
    # SP queue: the big weight stream, most-urgent first.
    whT = const.tile([128, NT * KB * 128], BF16, name="whT")
    whT8 = const.tile([128, NT * 2 * 128], FP8, name="whT8")
    wpos = const.tile([128, KB * A], BF16, name="wpos")
    wpos8 = const.tile([128, 2 * A], FP8, name="wpos8")
    whz = const.tile([128, NT * H], BF16, name="whz")
    went1 = const.tile([128, KE1 * A], BF16, name="went1")
    went2 = const.tile([128, KE2 * A], FP8, name="went2")

    CW = KB * 128  # whT bf16 columns per token tile

    def whT_dma(lo, hi):
        nc.sync.dma_start(whT[:, lo * CW:hi * CW], whT_d[:, lo * CW:hi * CW])

    # small loads first: tiny transfers, but the gather chain hangs off ends
    # (ACT-issued DMA faults the device, so everything goes via SP)
    ends = const.tile([2 * BL, 1], I32, name="ends")
    nc.sync.dma_start(ends[:], ends_d[:])
    te_sb = const.tile([T, H], BF16, name="te_sb")
    nc.sync.dma_start(te_sb[:], te_d[:])
    v_sb = const.tile([1, A], BF16, name="v_sb")
    nc.sync.dma_start(v_sb[:], v_d[:])
    nc.sync.dma_start(wpos[:, 0:3 * A], wpos_d[:, 0:3 * A])              # k0-2
    whT_dma(0, 1)
    nc.sync.dma_start(wpos[:, 3 * A:], wpos_d[:, 3 * A:])                # k3-6
    nc.sync.dma_start(wpos8[:], wpos8_d[:])
    QT8 = NT * 64  # quarter of whT8 columns (4 token tiles)
    nc.sync.dma_start(whT8[:, 0:QT8], whT8_d[:, 0:QT8])
    whT_dma(1, 2)
    nc.sync.dma_start(whT8[:, QT8:2 * QT8], whT8_d[:, QT8:2 * QT8])
    whT_dma(2, 3)
    nc.sync.dma_start(whT8[:, 2 * QT8:], whT8_d[:, 2 * QT8:])
    whT_dma(3, 4)
    import os
    _ORD = os.environ.get("DMA_ORD", "F")
    went_dmas = [
        lambda: nc.sync.dma_start(went1[:, 0:8 * A], went1_d[:, 0:8 * A]),
        lambda: nc.sync.dma_start(went1[:, 8 * A:], went1_d[:, 8 * A:]),
        lambda: nc.sync.dma_start(went2[:], went2_d[:]),
    ]
    whz_dmas = [
        lambda: nc.sync.dma_start(whz[:, 0:4 * H], whz_d[:, 0:4 * H]),
        lambda: nc.sync.dma_start(whz[:, 4 * H:8 * H], whz_d[:, 4 * H:8 * H]),
        lambda: nc.sync.dma_start(whz[:, 8 * H:12 * H], whz_d[:, 8 * H:12 * H]),
        lambda: nc.sync.dma_start(whz[:, 12 * H:], whz_d[:, 12 * H:]),
    ]
    whT_chunks = [lambda lo=lo: whT_dma(lo, lo + 2) for lo in range(4, 16, 2)]
    # each config: list of ('T', i) / ('E', i) / ('Z', i)
    ORDS = {
        # went all first (prev)
        "A": ["E0", "E1", "E2", "T0", "T1", "T2", "T3", "T4", "T5",
               "Z0", "Z1", "Z2", "Z3"],
        # went interleaved every other whT chunk
        "B": ["E0", "T0", "E1", "T1", "E2", "T2", "T3", "T4", "T5",
               "Z0", "Z1", "Z2", "Z3"],
        # went slightly later
        "C": ["T0", "E0", "T1", "E1", "T2", "E2", "T3", "T4", "T5",
               "Z0", "Z1", "Z2", "Z3"],
        # went2 first (group flip not needed: de_ty start flag handles)
        "D": ["E2", "T0", "E0", "T1", "E1", "T2", "T3", "T4", "T5",
               "Z0", "Z1", "Z2", "Z3"],
        # whz earlier, went mid
        "E": ["E0", "T0", "E1", "T1", "E2", "T2", "T3", "Z0", "T4",
               "Z1", "T5", "Z2", "Z3"],
        # big whT head: tiles 4-7 before went; rest JIT after
        "F": ["T0", "T1", "E0", "E1", "E2", "T2", "T3", "T4", "T5",
               "Z0", "Z1", "Z2", "Z3"],
        # same + whz interleaved among tail whT
        "G": ["T0", "T1", "E0", "E1", "E2", "T2", "Z0", "T3", "Z1",
               "T4", "Z2", "T5", "Z3"],
        # even bigger head
        "H": ["T0", "T1", "T2", "E0", "E1", "E2", "T3", "T4", "T5",
               "Z0", "Z1", "Z2", "Z3"],
        # went interleaved between whT chunks
        "I": ["T0", "T1", "E0", "T2", "E1", "T3", "E2", "T4", "T5",
               "Z0", "Z1", "Z2", "Z3"],
    }
    for tok in ORDS[_ORD]:
        kind, idx = tok[0], int(tok[1:])
        if kind == "T":
            whT_chunks[idx]()
        elif kind == "E":
            went_dmas[idx]()
        else:
            whz_dmas[idx]()



    # ---------- gather chain (gpsimd/DVE; latency-critical) ----------
    gidx = const.tile([2 * BL, 1], I32, name="gidx")
    nc.gpsimd.iota(gidx[:], pattern=[[1, 1]], base=0, channel_multiplier=L)
    nc.vector.tensor_scalar(out=gidx[:], in0=gidx[:], scalar1=BL * L - 1,
                            scalar2=None, op0=ALU.bitwise_and)
    nc.vector.tensor_tensor(out=gidx[:], in0=gidx[:], in1=ends[:], op=ALU.add)
    eh = const.tile([2 * BL, H], F32R, name="eh")
    nc.gpsimd.indirect_dma_start(
        out=eh[:], out_offset=None, in_=wh_d[:],
        in_offset=bass.IndirectOffsetOnAxis(ap=gidx[:, 0:1], axis=0))

    # ---------- other small device constants ----------
    iota_p = const.tile([128, 128], I32, name="iota_p")
    iota_f = const.tile([128, 128], I32, name="iota_f")
    nc.gpsimd.iota(iota_p[:], pattern=[[0, 128]], base=0, channel_multiplier=1)
    nc.gpsimd.iota(iota_f[:], pattern=[[1, 128]], base=0, channel_multiplier=0)
    ident = const.tile([128, 128], F32R, name="ident")
    nc.vector.tensor_tensor(out=ident[:], in0=iota_p[:], in1=iota_f[:],
                            op=ALU.is_equal)
    ident_bf = const.tile([128, 128], BF16, name="ident_bf")
    nc.vector.tensor_copy(out=ident_bf[:], in_=ident[:].bitcast(F32))
    ones_bf = const.tile([128, 1], BF16, name="ones_bf")
    nc.gpsimd.memset(ones_bf[:], 1.0)
    v_bc = const.tile([128, A], BF16, name="v_bc")
    nc.gpsimd.partition_broadcast(v_bc[:], v_sb[0:1, :])

    # ---------- PE program ----------
    # 0) heater: keep PE busy (and ramping) until real operands land.
    heat_ps = ps_sm.tile([128, HEAT_COLS], F32, name="heat_ps", tag="sm")
    for _ in range(N_HEAT):
        nc.tensor.matmul(heat_ps[0:HEAT_COLS, :], lhsT=hl[:], rhs=hl[:],
                         start=True, stop=True, skip_group_check=True)

    whT_v = whT.rearrange("p (i k c) -> p i k c", i=NT, k=KB)
    whT8_v = whT8.rearrange("p (i two c) -> p i two c", i=NT, two=2)
    wpos8_v = wpos8.rearrange("p (two a) -> p two a", two=2)
    wpos_v = wpos.rearrange("p (k a) -> p k a", k=KB)
    whz_v = whz.rearrange("p (i h) -> p i h", i=NT)
    went1_v = went1.rearrange("p (k a) -> p k a", k=KE1)
    went2_v = went2.rearrange("p (k a) -> p k a", k=KE2)

    teT = const.tile([128, HC * T], BF16, name="teT")
    efT = const.tile([128, 32 * BL], BF16, name="efT")
    acc = ps_acc.tile([128, 128], F32, name="acc", tag="acc")
    deT = acc[:, 0:4 * BL]
    zt = acc[:, 32:32 + HC * BL]
    esum_row = acc[0:1, 96:96 + BL]
    bias_sb = const.tile([128, 4 * BL], F32, name="bias_sb")
    vu = const.tile([128, NT], F32, name="vu")
    expb = const.tile([128, NT], BF16, name="expb")
    u_t = [None] * NT
    dp_t = [None] * NT

    steps = []          # (min_tile, thunk) — drained between dp k-matmuls

    def emit_dp(i, drain):
        dp = ps_dp.tile([128, A], F32, tag="dp", name=f"dp{i}")
        for k in range(KB):
            nc.tensor.matmul(dp[:], lhsT=whT_v[:, i, k, :], rhs=wpos_v[:, k, :],
                             start=(k == 0), stop=False)
            if i == 0 and k == 2:
                for _ in range(MID_HEAT):
                    nc.tensor.matmul(heat_ps[0:HEAT_COLS, :], lhsT=hl[:],
                                     rhs=hl[:], start=True, stop=True,
                                     skip_group_check=True)
            drain(i)
        if _os.environ.get("NODR"):
            nc.tensor.matmul(dp[:], lhsT=whT8_v[:, i, 0, :], rhs=wpos8_v[:, 0, :],
                             start=False, stop=False)
            nc.tensor.matmul(dp[:], lhsT=whT8_v[:, i, 1, :], rhs=wpos8_v[:, 1, :],
                             start=False, stop=True)
        else:
            nc.tensor.matmul(dp[:], lhsT=whT8_v[:, i, :, :], rhs=wpos8_v[:],
                             start=False, stop=True,
                             perf_mode=mybir.MatmulPerfMode.DoubleRow)
        drain(i)
        if i < NCOPY:
            dpc = const.tile([128, A], F32, name=f"dpc{i}")
            if i % 2 == 0:
                nc.vector.tensor_copy(out=dpc[:], in_=dp[:])
            else:
                nc.scalar.copy(dpc[:], dp[:])
            dp_t[i] = dpc
        else:
            dp_t[i] = dp

    def queue_entity_gather():
        def tr(hc):
            def f():
                pt = ps_sm.tile([128, 128], F32R, tag="sm", name=f"ehT{hc}")
                nc.tensor.transpose(pt[:, 0:2 * BL], eh[:, hc * 128:(hc + 1) * 128],
                                    ident[0:2 * BL, 0:2 * BL])
                nc.vector.tensor_copy(out=efT[:, hc * BL:(hc + 1) * BL],
                                      in_=pt[:, 0:BL].bitcast(F32))
                nc.vector.tensor_copy(out=efT[:, (HC + hc) * BL:(HC + hc + 1) * BL],
                                      in_=pt[:, BL:2 * BL].bitcast(F32))
            return f
        def trte(hc):
            def f():
                pt = ps_sm.tile([128, 128], F32R, tag="sm", name=f"teT{hc}")
                ptb = pt.bitcast(BF16)
                nc.tensor.transpose(ptb[:, 0:T], te_sb[:, hc * 128:(hc + 1) * 128],
                                    ident_bf[0:T, 0:T])
                nc.vector.tensor_copy(out=teT[:, hc * T:(hc + 1) * T],
                                      in_=ptb[:, 0:T])
            return f
        for hc in range(HC):
            steps.append((E_GATHER, tr(hc)))
        for hc in range(HC):
            steps.append((E_GATHER, trte(hc)))

    def queue_de(kts, base, w_v, min_tile):
        # dense_ent k-tiles: 2 matmuls per step; every column region resets
        # on the overall first k-tile and closes on the overall last one.
        def mk(kt, s0):
            def f():
                for s in (s0, s0 + 1):
                    nc.tensor.matmul(deT[:, s * BL:(s + 1) * BL],
                                     lhsT=w_v[:, kt, s * 128:(s + 1) * 128],
                                     rhs=efT[:, (base + kt) * BL:(base + kt + 1) * BL],
                                     start=(base + kt == 0 and s == 0),
                                     stop=(base + kt == KE1 + KE2 - 1 and s == 3),
                                     skip_group_check=True)
            return f
        for kt in range(kts):
            steps.append((min_tile, mk(kt, 0)))
            steps.append((min_tile, mk(kt, 2)))

    def queue_type_chain():
        def sc_mk(ent, hc0):
            def f():
                sc = sc_t[ent]
                for hc in (hc0, hc0 + 1):
                    col = (0 if ent == 0 else HC) + hc
                    nc.tensor.matmul(sc[:], lhsT=efT[:, col * BL:(col + 1) * BL],
                                     rhs=teT[:, hc * T:(hc + 1) * T],
                                     start=(hc == 0), stop=(hc == HC - 1))
            return f
        def soft_mk(ent):
            def f():
                sc = sc_t[ent]
                asm = const.tile([BL, T], F32, name=f"asm{ent}")
                ssum = const.tile([BL, 1], F32, name=f"ssum{ent}")
                nc.scalar.activation(asm[:], sc[:], AF.Exp, accum_out=ssum[:])
                rs = const.tile([BL, 1], F32, name=f"rs{ent}")
                nc.vector.reciprocal(rs[:], ssum[:])
                al = const.tile([BL, T], F32R, name=f"al{ent}")
                nc.vector.tensor_scalar(out=al[:], in0=asm[:], scalar1=rs[:, 0:1],
                                        scalar2=None, op0=ALU.mult)
                al_t[ent] = al
            return f
        def alt_mk(ent):
            def f():
                pt = ps_sm.tile([128, 128], F32R, tag="sm", name=f"alT{ent}")
                nc.tensor.transpose(pt[0:T, 0:BL], al_t[ent][:], ident[0:BL, 0:BL])
                alTe = const.tile([T, BL], BF16, name=f"alTe{ent}")
                nc.vector.tensor_copy(out=alTe[:], in_=pt[0:T, 0:BL].bitcast(F32))
                alTe_t[ent] = alTe
            return f
        def pet_mk(ent, hc0):
            def f():
                base = 16 if ent == 0 else 24
                for hc in (hc0, hc0 + 1):
                    pe = ps_sm.tile([128, BL], F32, tag="sm", name=f"pet{ent}{hc}")
                    nc.tensor.matmul(pe[:], lhsT=te_sb[:, hc * 128:(hc + 1) * 128],
                                     rhs=alTe_t[ent][:], start=True, stop=True)
                    nc.vector.tensor_copy(
                        out=efT[:, (base + hc) * BL:(base + hc + 1) * BL], in_=pe[:])
            return f
        for ent in range(2):
            for hc0 in range(0, HC, 2):
                steps.append((E_TYPE, sc_mk(ent, hc0)))
            steps.append((E_TYPE, soft_mk(ent)))
            steps.append((E_TYPE, alt_mk(ent)))
            for hc0 in range(0, HC, 2):
                steps.append((E_TYPE, pet_mk(ent, hc0)))

    def queue_bias_copy():
        def f():
            nc.vector.tensor_copy(out=bias_sb[:], in_=deT[:])
        steps.append((E_DE, f))

    def emit_tanh_chain(j):
        b, h = j // 2, j % 2
        u = work.tile([128, A], BF16, tag="u", name=f"u{j}")
        nc.scalar.activation(u[:, 0:256], dp_t[j][:, 0:256], AF.Tanh,
                             bias=bias_sb[:, (2 * h) * BL + b:(2 * h) * BL + b + 1])
        nc.scalar.activation(u[:, 256:512], dp_t[j][:, 256:512], AF.Tanh,
                             bias=bias_sb[:, (2 * h + 1) * BL + b:(2 * h + 1) * BL + b + 1])
        u_t[j] = u
        scr = work.tile([128, A], BF16, tag="scr", name=f"scr{j}")
        nc.vector.tensor_tensor(out=scr[:], in0=u[:], in1=v_bc[:], op=ALU.mult)
        nc.vector.tensor_reduce(out=vu[:, j:j + 1], in_=scr[:],
                                axis=mybir.AxisListType.X, op=ALU.add)

    def emit_z(j):
        # one PSUM group for the whole acc bank: start only on the very
        # first matmul of the drain; pending-zero covers every region
        b = j // 2
        nc.tensor.matmul(esum_row[0:1, b:b + 1], lhsT=expb[:, j:j + 1],
                         rhs=ones_bf[:], start=(j == 0), stop=False,
                         skip_group_check=True)
        for s in range(HC):
            nc.tensor.matmul(zt[:, s * BL + b:s * BL + b + 1],
                             lhsT=whz_v[:, j, s * 128:(s + 1) * 128],
                             rhs=expb[:, j:j + 1],
                             start=False,
                             stop=(j == NT - 1 and s == HC - 1),
                             skip_group_check=True)

    # queue all side work (PE bits chopped <=2 instrs so the 4-deep
    # wait-queue never clogs; deps gate execution)
    sc_t = [None, None]
    al_t = [None, None]
    alTe_t = [None, None]
    sc_t[0] = ps_sm.tile([BL, T], F32, tag="sm", name="sc0")
    sc_t[1] = ps_sm.tile([BL, T], F32, tag="sm", name="sc1")
    queue_entity_gather()
    queue_type_chain()
    queue_de(KE1, 0, went1_v, E_GATHER)
    queue_de(KE2, KE1, went2_v, E_DE)
    queue_bias_copy()

    sp = [0]

    def drain(i):
        n = 0
        while sp[0] < len(steps) and steps[sp[0]][0] <= i and n < 2:
            steps[sp[0]][1]()
            sp[0] += 1
            n += 1

    done_tanh = 0
    done_exp = 0

    def pump_exp():
        nonlocal done_exp
        while done_exp + 2 <= done_tanh - 2:
            j = done_exp
            nc.scalar.activation(expb[:, j:j + 2], vu[:, j:j + 2], AF.Exp)
            done_exp += 2

    for i in range(NT):
        emit_dp(i, drain)
        if i > E_DE + 1:
            while done_tanh <= i - 1:
                emit_tanh_chain(done_tanh)
                done_tanh += 1
                pump_exp()
    while sp[0] < len(steps):
        steps[sp[0]][1]()
        sp[0] += 1
    while done_tanh < NT:
        emit_tanh_chain(done_tanh)
        done_tanh += 1
        pump_exp()
    while done_exp < NT:
        j = done_exp
        nc.scalar.activation(expb[:, j:j + 2], vu[:, j:j + 2], AF.Exp)
        done_exp += 2

    # ---------- z drain + split epilogue (batches 0-5 stored early) ----------
    rec_row = const.tile([1, BL], F32, name="rec_row")
    rec_bc = const.tile([128, BL], F32, name="rec_bc")
    z_sb = const.tile([128, HC * BL], F32, name="z_sb")
    out_v = out_d.rearrange("(s p) b -> p s b", p=128)

    def emit_store(b0, b1):
        nb = b1 - b0
        nc.vector.reciprocal(rec_row[:, b0:b1], esum_row[:, b0:b1])
        nc.gpsimd.partition_broadcast(rec_bc[:, b0:b1], rec_row[0:1, b0:b1])
        rb_v = rec_bc[:, b0:b1].rearrange(
            "p (s b) -> p s b", s=1).broadcast_to([128, HC, nb])
        zt_v = zt.rearrange("p (s b) -> p s b", s=HC)[:, :, b0:b1]
        zs_v = z_sb.rearrange("p (s b) -> p s b", s=HC)[:, :, b0:b1]
        nc.vector.tensor_tensor(out=zs_v, in0=zt_v, in1=rb_v, op=ALU.mult)
        nc.sync.dma_start(out_v[:, :, b0:b1], zs_v)

    if _os.environ.get("ONESTORE"):
        for j in range(NT):
            emit_z(j)
        emit_store(0, BL)
    else:
        for j in range(12):
            emit_z(j)
        emit_store(0, 6)
        for j in range(12, NT):
            emit_z(j)
        emit_store(6, BL)

    if _os.environ.get("DBG"):
        dbg_bias = nc.dram_tensor("dbg_bias", [128, 32], F32, kind="ExternalOutput").ap()
        nc.sync.dma_start(dbg_bias[:], bias_sb[:])
        dbg_vu = nc.dram_tensor("dbg_vu", [128, NT], F32, kind="ExternalOutput").ap()
        nc.sync.dma_start(dbg_vu[:], vu[:])
        dbg_dp = nc.dram_tensor("dbg_dp", [128, A], F32, kind="ExternalOutput").ap()
        nc.sync.dma_start(dbg_dp[:], dp_t[0][:])
        dbg_ef = nc.dram_tensor("dbg_ef", [128, 256], F32, kind="ExternalOutput").ap()
        efc = const.tile([128, 256], F32, name="efc")
        nc.vector.tensor_copy(out=efc[:], in_=efT[:])
        nc.sync.dma_start(dbg_ef[:], efc[:])
    for p in (ps_acc, ps_sm, ps_dp, work, const):
        p.release()


def build():
    nc = bacc.Bacc("TRN2", target_bir_lowering=False, debug=False,
                   num_devices=NCORES)
    with tile.TileContext(nc) as tc:
        _build_core(tc)
    nc.compile()
    return nc


_NC = None


def _colperm():
    j = np.arange(A)
    s, p = j // 128, j % 128
    return 256 * (s // 2) + 2 * p + (s % 2)


def kernel(word_hiddens, pos_e1_embeddings, pos_e2_embeddings, e1_end, e2_end,
           type_embeddings, W_pos, W_ent, v):
    global _NC
    if _NC is None:
        _NC = build()
    BF = ml_dtypes.bfloat16
    F8 = ml_dtypes.float8_e4m3
    wh = np.ascontiguousarray(word_hiddens, dtype=np.float32).reshape(B, L, H)
    p1 = np.ascontiguousarray(pos_e1_embeddings, dtype=np.float32).reshape(B, L, P2)
    p2 = np.ascontiguousarray(pos_e2_embeddings, dtype=np.float32).reshape(B, L, P2)
    e1 = np.asarray(e1_end, dtype=np.int32).reshape(B)
    e2 = np.asarray(e2_end, dtype=np.int32).reshape(B)
    te = np.ascontiguousarray(type_embeddings, dtype=np.float32).astype(BF)
    perm = _colperm()
    we0 = np.asarray(W_ent, dtype=np.float32).reshape(4, H, A)
    went1 = np.concatenate([we0[0], we0[2]], axis=0)[:, perm]
    went2 = np.concatenate([we0[1], we0[3]], axis=0)[:, perm]
    went1 = np.ascontiguousarray(
        went1.reshape(KE1, 128, A).transpose(1, 0, 2).reshape(128, KE1 * A)).astype(BF)
    went2 = np.ascontiguousarray(
        went2.reshape(KE2, 128, A).transpose(1, 0, 2).reshape(128, KE2 * A)).astype(F8)
    wp = np.asarray(W_pos, dtype=np.float32)
    wpk = wp.reshape(KF, 128, A)
    wpos = np.ascontiguousarray(
        wpk[:KB].transpose(1, 0, 2).reshape(128, KB * A)).astype(BF)
    wpos8 = np.ascontiguousarray(
        wpk[KB:].transpose(1, 0, 2).reshape(128, 2 * A)).astype(F8)
    vv = np.ascontiguousarray(v, dtype=np.float32).reshape(1, A).astype(BF)

    in_maps = []
    for c in range(NCORES):
        s = slice(c * BL, (c + 1) * BL)
        whc = np.ascontiguousarray(wh[s].reshape(TOK, H))
        pf = np.empty((TOK, F), dtype=np.float32)
        pf[:, :H] = whc
        pf[:, H:H + P2] = p1[s].reshape(TOK, P2)
        pf[:, H + P2:] = p2[s].reshape(TOK, P2)
        pfk = pf.reshape(NT, 128, KF, 128)
        whT = np.ascontiguousarray(
            pfk[:, :, :KB].transpose(3, 0, 2, 1)
            .reshape(128, NT * KB * 128)).astype(BF)
        whT8 = np.ascontiguousarray(
            pfk[:, :, KB:].transpose(3, 0, 2, 1)
            .reshape(128, NT * 2 * 128)).astype(F8)
        whz = np.ascontiguousarray(
            whc.reshape(NT, 128, H).transpose(1, 0, 2).reshape(128, NT * H)).astype(BF)
        ends = np.concatenate([e1[s], e2[s]]).reshape(2 * BL, 1)
        in_maps.append({
            "whT": whT,
            "whT8": whT8,
            "W_pos8": wpos8,
            "whz": whz,
            "W_pos": wpos,
            "went1": went1,
            "went2": went2,
            "word_hiddens": whc,
            "ends": np.ascontiguousarray(ends),
            "type_embeddings": te,
            "v": vv,
        })
    res = run_bass_kernel_spmd(_NC, in_maps, core_ids=list(range(NCORES)))
    return np.concatenate(
        [res.results[c]["out"].T for c in range(NCORES)], axis=0).astype(np.float32)
